# revision 1
# baseline (speedup 1.0000x reference)
"""GQA kernel for Trainium2, 8 NeuronCores.

Sharding: core c = b*4 + kv  (b in {0,1} data-parallel over batch,
kv in {0..3} tensor-parallel over the 4 KV head groups; each core owns
4 Q heads + 1 KV head). Each core computes a partial output
x[b] @ Wq[:,kv] -> attention -> @ Wo[kv rows]; host sums the 4 partials
per batch (the row-sharded-Wo all-reduce).

Device layout (per core): everything keyed off transposed activations
xT = x[b].T so all matmuls keep the contraction on the partition dim and
N=256/512 moving operands (fp32r full-rate):
  QT[d,t] = Wq_h^T x^T   (norm+rope applied in-layout)
  S^T[k,q] = K Q^T       (lhsT = K^T tile)
  P = exp(S^T/sqrt(d)) * causal_mask     (no max-subtraction needed:
                                          |S|<=sqrt(d) after RMSNorm)
  L[q] (softmax denom) via ones-matmul (partition reduction + broadcast)
  O^T[d,q] = V^T... (lhsT = V natural [k,d]) accumulated over k blocks
  out_partial[q,:] = O^T.T @ Wo  with 1/L folded in before Wo.
"""

import numpy as np

B, T, D = 2, 2048, 2048
NH, NKV, HD = 16, 4, 128
GQ = NH // NKV            # 4 q heads per kv head
HQ = GQ * HD              # 512 q-dim per core
ROPE_BASE = 500000.0
EPS = 1e-5
SCALE = 1.0 / np.sqrt(HD)
NE = 8                    # phase-1 T eighths
ET = T // NE              # 256
NDC = D // 128            # 16 contraction chunks
NJ = 4                    # phase-2 q chunks of 512
JW = T // NJ              # 512
NKB = T // 128            # 16 k blocks

_cached = {}


def _build_program():
    import concourse.bacc as bacc
    import concourse.mybir as mybir
    from concourse import tile
    from concourse.masks import make_identity

    f32 = mybir.dt.float32
    f32r = mybir.dt.float32r
    AF = mybir.ActivationFunctionType
    from concourse.bass import ts

    nc = bacc.Bacc()

    xt = nc.dram_tensor("xt", [D, T], f32r, kind="ExternalInput")
    wq = nc.dram_tensor("wq", [D, HQ], f32r, kind="ExternalInput")
    wk = nc.dram_tensor("wk", [D, HD], f32r, kind="ExternalInput")
    wv = nc.dram_tensor("wv", [D, HD], f32r, kind="ExternalInput")
    wo = nc.dram_tensor("wo", [HQ, D], f32r, kind="ExternalInput")
    cosd = nc.dram_tensor("cos", [HD, T], f32, kind="ExternalInput")
    sind = nc.dram_tensor("sin", [HD, T], f32, kind="ExternalInput")
    wqcd = nc.dram_tensor("wqc", [HD, 1], f32, kind="ExternalInput")
    wkcd = nc.dram_tensor("wkc", [HD, 1], f32, kind="ExternalInput")
    wqed = nc.dram_tensor("wqe", [HD, 1], f32, kind="ExternalInput")
    wked = nc.dram_tensor("wke", [HD, 1], f32, kind="ExternalInput")
    mskd = nc.dram_tensor("msk", [4, 128, JW], f32r, kind="ExternalInput")
    onesd = nc.dram_tensor("ones", [128, 128], f32r, kind="ExternalInput")
    onesnd = nc.dram_tensor("onesn", [128, 128], f32r, kind="ExternalInput")
    outd = nc.dram_tensor("out", [T, D], f32, kind="ExternalOutput")

    xtr = xt.rearrange("(c p) t -> p c t", p=128)
    wqr = wq.rearrange("(c p) n -> p c n", p=128)
    wkr = wk.rearrange("(c p) n -> p c n", p=128)
    wvr = wv.rearrange("(c p) n -> p c n", p=128)
    wor = wo.rearrange("(c p) n -> p c n", p=128)

    with tile.TileContext(nc) as tc:
        with tc.tile_pool(name="A", bufs=1) as A:
            # persistent across all phases
            QT = A.tile([128, GQ, T], f32r, tag="QT")
            KT = A.tile([128, T], f32r, tag="KT")
            Vn = A.tile([128, NKB, HD], f32r, tag="Vn")
            msk_sb = A.tile([128, 4, JW], f32r, tag="msk")
            ones_sb = A.tile([128, 128], f32r, tag="ones")
            ident = A.tile([128, 128], f32, tag="ident")
            onesn_sb = A.tile([128, 128], f32r, tag="onesn")
            eps_t = A.tile([128, 1], f32, tag="eps")
            nc.vector.memset(eps_t, EPS)
            nc.sync.dma_start(out=ones_sb, in_=onesd[:, :])
            nc.sync.dma_start(out=onesn_sb, in_=onesnd[:, :])
            nc.sync.dma_start(out=msk_sb, in_=mskd.rearrange("r p q -> p r q"))
            make_identity(nc, ident)

            with (
                tc.tile_pool(name="W", bufs=1) as W,
                tc.tile_pool(name="Bp", bufs=1) as Bp,
                tc.tile_pool(name="BX", bufs=2) as BX,
                tc.tile_pool(name="TMP", bufs=3) as TMP,
                tc.tile_pool(name="PS1", bufs=3, space="PSUM") as PS1,
                tc.tile_pool(name="PS1b", bufs=2, space="PSUM") as PS1b,
            ):
                wq_sb = W.tile([128, NDC, HQ], f32r, tag="wq")
                wk_sb = W.tile([128, NDC, HD], f32r, tag="wk")
                wv_sb = W.tile([128, NDC, HD], f32r, tag="wv")
                xt_e0 = BX.tile([128, NDC, ET], f32r, tag="xt")
                for c in range(NDC):
                    nc.sync.dma_start(out=wk_sb[:, c, :], in_=wkr[:, c, :])
                    nc.sync.dma_start(out=xt_e0[:, c, :], in_=xtr[:, c, 0:ET])
                    nc.sync.dma_start(out=wv_sb[:, c, :], in_=wvr[:, c, :])
                    nc.sync.dma_start(out=wq_sb[:, c, :], in_=wqr[:, c, :])
                cos_sb = Bp.tile([128, T], f32, tag="cos")
                sin_sb = Bp.tile([128, T], f32, tag="sin")
                nc.sync.dma_start(out=cos_sb, in_=cosd[:, :])
                nc.sync.dma_start(out=sin_sb, in_=sind[:, :])
                wqc = Bp.tile([128, 1], f32, tag="wqc")
                wkc = Bp.tile([128, 1], f32, tag="wkc")
                wqe = Bp.tile([128, 1], f32, tag="wqe")
                wke = Bp.tile([128, 1], f32, tag="wke")
                nc.sync.dma_start(out=wqc, in_=wqcd[:, :])
                nc.sync.dma_start(out=wkc, in_=wkcd[:, :])
                nc.sync.dma_start(out=wqe, in_=wqed[:, :])
                nc.sync.dma_start(out=wke, in_=wked[:, :])

                def normrope(cpsum, wcol, wbias, sl, out_sl):
                    """RMSNorm (over partition dim via ones-matmul broadcast)
                    + norm-weight + RoPE; writes f32r out_sl [128, ET]."""
                    sq = TMP.tile([128, ET], f32r, tag="sq")
                    nc.scalar.activation(sq, cpsum, AF.Square)
                    l2 = PS1b.tile([128, ET], f32, tag="l2")
                    nc.tensor.matmul(l2, onesn_sb, sq, start=True, stop=True)
                    sv = TMP.tile([128, ET], f32, tag="sv")
                    nc.scalar.activation(sv, l2, AF.Sqrt, scale=wcol, bias=wbias)
                    rc = TMP.tile([128, ET], f32, tag="rc")
                    nc.vector.reciprocal(rc, sv)
                    qn = TMP.tile([128, ET], f32, tag="qn")
                    nc.vector.tensor_mul(qn, cpsum, rc)
                    qr = TMP.tile([128, ET], f32, tag="qr")
                    nc.sync.dma_start(out=qr[:64], in_=qn[64:])
                    nc.sync.dma_start(out=qr[64:], in_=qn[:64])
                    t1 = TMP.tile([128, ET], f32, tag="t1")
                    nc.vector.tensor_mul(t1, qn, cos_sb[:, sl])
                    t2 = TMP.tile([128, ET], f32, tag="t2")
                    nc.vector.tensor_mul(t2, qr, sin_sb[:, sl])
                    nc.vector.tensor_sub(out_sl[:64], t1[:64], t2[:64])
                    nc.vector.tensor_add(out_sl[64:], t1[64:], t2[64:])

                for e in range(NE):
                    sl = ts(e, ET)
                    if e == 0:
                        xt_t = xt_e0
                    else:
                        xt_t = BX.tile([128, NDC, ET], f32r, tag="xt")
                        nc.sync.dma_start(out=xt_t, in_=xtr[:, :, sl])
                    # K eighth
                    kp = PS1.tile([128, ET], f32, tag="pp")
                    for c in range(NDC):
                        nc.tensor.matmul(kp, wk_sb[:, c, :], xt_t[:, c, :],
                                         start=(c == 0), stop=(c == NDC - 1))
                    normrope(kp, wkc, wke, sl, KT[:, sl])
                    # V eighth: project transposed then PE-transpose to natural
                    vp = PS1.tile([128, ET], f32, tag="pp")
                    for c in range(NDC):
                        nc.tensor.matmul(vp, wv_sb[:, c, :], xt_t[:, c, :],
                                         start=(c == 0), stop=(c == NDC - 1))
                    vt = TMP.tile([128, ET], f32, tag="vt")
                    nc.scalar.activation(vt, vp, AF.Copy)
                    for i in range(ET // 128):
                        tp = PS1b.tile([128, 128], f32, tag="tp")
                        nc.tensor.transpose(tp, vt[:, ts(i, 128)], ident)
                        nc.scalar.activation(Vn[:, e * (ET // 128) + i, :], tp, AF.Copy)
                    # Q heads
                    for h in range(GQ):
                        qp = PS1.tile([128, ET], f32, tag="pp")
                        for c in range(NDC):
                            nc.tensor.matmul(qp, wq_sb[:, c, ts(h, 128)], xt_t[:, c, :],
                                             start=(c == 0), stop=(c == NDC - 1))
                        normrope(qp, wqc, wqe, sl, QT[:, h, sl])

            with (
                tc.tile_pool(name="C", bufs=1) as C,
                tc.tile_pool(name="CP", bufs=4) as CP,
                tc.tile_pool(name="CT", bufs=3) as CT,
                tc.tile_pool(name="CO", bufs=2) as CO,
            ):
                OT = C.tile([128, GQ, T], f32r, tag="OT")
                wo_sb = C.tile([128, GQ, D], f32r, tag="wo")
                for c in range(GQ):
                    nc.sync.dma_start(out=wo_sb[:, c, :], in_=wor[:, c, :])

                with (
                    tc.tile_pool(name="PS2", bufs=2, space="PSUM") as PS2,
                    tc.tile_pool(name="PS2b", bufs=2, space="PSUM") as PS2b,
                    tc.tile_pool(name="PS3", bufs=2, space="PSUM") as PS3,
                ):
                    for J in range(NJ):
                        nkb = 4 * J + 4
                        for h in range(GQ):
                            lp = PS2b.tile([128, JW], f32, tag="l")
                            op = PS2b.tile([128, JW], f32, tag="ot")
                            for kb in range(nkb):
                                sp = PS2.tile([128, JW], f32, tag="s")
                                nc.tensor.matmul(sp, KT[:, ts(kb, 128)],
                                                 QT[:, h, ts(J, JW)],
                                                 start=True, stop=True)
                                P = CP.tile([128, JW], f32r, tag="p")
                                nc.scalar.activation(P, sp, AF.Exp, scale=SCALE)
                                if kb >= 4 * J:
                                    nc.vector.tensor_mul(P, P, msk_sb[:, kb - 4 * J, :])
                                nc.tensor.matmul(lp, ones_sb, P,
                                                 start=(kb == 0), stop=(kb == nkb - 1))
                                nc.tensor.matmul(op, Vn[:, kb, :], P,
                                                 start=(kb == 0), stop=(kb == nkb - 1))
                            rc2 = CT.tile([128, JW], f32, tag="rc2")
                            nc.vector.reciprocal(rc2, lp)
                            nc.vector.tensor_mul(OT[:, h, ts(J, JW)], op, rc2)
                        # output projection for this J's four q-tiles (overlaps next J)
                        for qt in range(4 * J, 4 * J + 4):
                            ost = CO.tile([128, D], f32, tag="ost")
                            for c in range(D // JW):
                                oup = PS3.tile([128, JW], f32, tag="op")
                                for hc in range(GQ):
                                    nc.tensor.matmul(oup, OT[:, hc, ts(qt, 128)],
                                                     wo_sb[:, hc, ts(c, JW)],
                                                     start=(hc == 0), stop=(hc == GQ - 1))
                                nc.scalar.activation(ost[:, ts(c, JW)], oup, AF.Copy)
                            nc.sync.dma_start(out=outd[qt * 128:(qt + 1) * 128, :], in_=ost)

    nc.finalize()
    return nc


def _host_consts():
    inv = 1.0 / (ROPE_BASE ** (np.arange(0, HD, 2, dtype=np.float64) / HD))
    freqs = np.outer(np.arange(T, dtype=np.float64), inv)
    emb = np.concatenate([freqs, freqs], axis=-1)          # [T, HD]
    cosT = np.ascontiguousarray(np.cos(emb).T.astype(np.float32))  # [HD, T]
    sinT = np.ascontiguousarray(np.sin(emb).T.astype(np.float32))
    msk = np.zeros((4, 128, JW), np.float32)
    for r in range(4):
        k = np.arange(128)[:, None] + 128 * r
        q = np.arange(JW)[None, :]
        msk[r] = (k <= q).astype(np.float32)
    ones = np.ones((128, 128), np.float32)
    return cosT, sinT, msk, ones


def kernel(x, Wq, Wk, Wv, Wo, q_norm_w, k_norm_w):
    from concourse.bass_utils import run_bass_kernel_spmd

    if "nc" not in _cached:
        _cached["nc"] = _build_program()
        _cached["consts"] = _host_consts()
    nc = _cached["nc"]
    cosT, sinT, msk, ones = _cached["consts"]

    x = np.asarray(x, np.float32)
    Wq = np.asarray(Wq, np.float32)
    Wk = np.asarray(Wk, np.float32)
    Wv = np.asarray(Wv, np.float32)
    Wo = np.asarray(Wo, np.float32)
    qwf = np.asarray(q_norm_w, np.float64).reshape(HD, 1)
    kwf = np.asarray(k_norm_w, np.float64).reshape(HD, 1)
    qw = np.ascontiguousarray((1.0 / qwf ** 2).astype(np.float32))
    kw = np.ascontiguousarray((1.0 / kwf ** 2).astype(np.float32))
    qwe = np.ascontiguousarray((EPS / qwf ** 2).astype(np.float32))
    kwe = np.ascontiguousarray((EPS / kwf ** 2).astype(np.float32))

    xTb = [np.ascontiguousarray(x[b].T) for b in range(B)]
    in_maps = []
    for core in range(8):
        b, kv = divmod(core, NKV)
        in_maps.append({
            "xt": xTb[b],
            "wq": np.ascontiguousarray(Wq[:, kv * HQ:(kv + 1) * HQ]),
            "wk": np.ascontiguousarray(Wk[:, kv * HD:(kv + 1) * HD]),
            "wv": np.ascontiguousarray(Wv[:, kv * HD:(kv + 1) * HD]),
            "wo": np.ascontiguousarray(Wo[kv * HQ:(kv + 1) * HQ, :]),
            "cos": cosT, "sin": sinT, "wqc": qw, "wkc": kw, "wqe": qwe, "wke": kwe,
            "msk": msk, "ones": ones, "onesn": ones / HD,
        })
    res = run_bass_kernel_spmd(nc, in_maps, list(range(8)))
    out = np.zeros((B, T, D), np.float64)
    for core in range(8):
        b = core // NKV
        out[b] += res.results[core]["out"].astype(np.float64)
    return out.astype(np.float32)



# revision 29
# speedup vs baseline: 1.3019x; 1.3019x over previous
"""GQA kernel for Trainium2, 8 NeuronCores.

Sharding: core c = b*4 + kv  (b in {0,1} data-parallel over batch,
kv in {0..3} tensor-parallel over the 4 KV head groups; each core owns
4 Q heads + 1 KV head). Each core computes a partial output
x[b] @ Wq[:,kv] -> attention -> @ Wo[kv rows]; host sums the 4 partials
per batch (the row-sharded-Wo all-reduce).

Device layout (per core), bf16 SBUF operands, f32 PSUM accumulation:
  phase 1 (per 512-col quarter of T, per head):
    KT/QT[d,t] = W^T x^T (contraction on partitions, N=512 moving).
    RMSNorm via ones-matmul partition reduction + Act Sqrt with the
    norm weight folded into scale/bias; RoPE via partition-swap DMA
    (SWDGE on the idle gpsimd queue) + DVE bf16 muls.
    Vn (natural [k,d]) via PE transpose.
  phase 2 (per q-slab J of 256, per head, kb groups of <=4 blocks):
    group: S^T = K Q^T (N=256 matmuls into a 2-bank PSUM tile), one
    Act exp over up to [128,1024] -> P bf16, DVE mask-mul on the
    diagonal tail, AV accumulation op += Vn^T P.
    Softmax denominator: near-free N=1 matmuls L[:,qc] += P_chunk^T
    @ ones (P is lhsT; L shares the op PSUM bank), then reciprocal ->
    PE transpose -> selector-matmul broadcast -> one DVE scale mul
    into OT. Out-projection chunks are interleaved between attention
    groups to keep PE saturated; output stores go out over SWDGE.
"""

import numpy as np

B, T, D = 2, 2048, 2048
NH, NKV, HD = 16, 4, 128
GQ = NH // NKV            # 4 q heads per kv head
HQ = GQ * HD              # 512 q-dim per core
ROPE_BASE = 500000.0
EPS = 1e-5
SCALE = 1.0 / np.sqrt(HD)
NQU = 4                   # phase-1 T quarters
QT_W = T // NQU           # 512
NDC = D // 128            # 16 contraction chunks
NJ = 8                    # phase-2 q slabs
JW = T // NJ              # 256
NKB = T // 128            # 16 k blocks
OCH = 512                 # out-projection D chunk

_cached = {}


def _build_program():
    import concourse.bacc as bacc
    import concourse.mybir as mybir
    from concourse import tile
    from concourse.masks import make_identity

    f32 = mybir.dt.float32
    f32r = mybir.dt.float32r
    bf16 = mybir.dt.bfloat16
    AF = mybir.ActivationFunctionType
    from concourse.bass import ts

    nc = bacc.Bacc()

    xt = nc.dram_tensor("xt", [D, T], bf16, kind="ExternalInput")
    wq = nc.dram_tensor("wq", [D, HQ], bf16, kind="ExternalInput")
    wk = nc.dram_tensor("wk", [D, HD], bf16, kind="ExternalInput")
    wv = nc.dram_tensor("wv", [D, HD], bf16, kind="ExternalInput")
    wo = nc.dram_tensor("wo", [HQ, D], bf16, kind="ExternalInput")
    cosd = nc.dram_tensor("cos", [HD, T], bf16, kind="ExternalInput")
    sind = nc.dram_tensor("sin", [HD, T], bf16, kind="ExternalInput")
    wqcd = nc.dram_tensor("wqc", [HD, 1], f32, kind="ExternalInput")
    wkcd = nc.dram_tensor("wkc", [HD, 1], f32, kind="ExternalInput")
    wqed = nc.dram_tensor("wqe", [HD, 1], f32, kind="ExternalInput")
    wked = nc.dram_tensor("wke", [HD, 1], f32, kind="ExternalInput")
    mskd = nc.dram_tensor("msk2", [HD, 2 * JW], bf16, kind="ExternalInput")
    onesnd = nc.dram_tensor("onesn", [128, 128], f32r, kind="ExternalInput")
    onescd = nc.dram_tensor("onesc", [128, 1], bf16, kind="ExternalInput")
    eseld = nc.dram_tensor("esel", [2, JW], bf16, kind="ExternalInput")
    outd = nc.dram_tensor("out", [T, D], f32, kind="ExternalOutput")

    xtr = xt.rearrange("(c p) t -> p c t", p=128)
    wqr = wq.rearrange("(c p) n -> p c n", p=128)
    wkr = wk.rearrange("(c p) n -> p c n", p=128)
    wvr = wv.rearrange("(c p) n -> p c n", p=128)
    wor = wo.rearrange("(c p) n -> p c n", p=128)
    mskr = mskd.rearrange("p (a q) -> p a q", a=2)

    with nc.allow_low_precision(reason="bf16 kernel, tolerance 2e-2"), \
         tile.TileContext(nc) as tc:
        with tc.tile_pool(name="A", bufs=1) as A, \
             tc.tile_pool(name="W", bufs=1) as W, \
             tc.tile_pool(name="BX", bufs=2) as BX:
            # persistent tensors and weights
            QT = A.tile([128, GQ, T], bf16, tag="QT")
            KT = A.tile([128, 1, T], bf16, tag="KT")
            Vn = A.tile([128, NKB, HD], bf16, tag="Vn")
            OT = A.tile([128, GQ, T], bf16, tag="OT")
            msk_sb = A.tile([128, 2, JW], bf16, tag="msk")
            onesn_sb = A.tile([128, 128], f32r, tag="onesn")
            onesc_sb = A.tile([128, 1], bf16, tag="onesc")
            esel_sb = A.tile([2, JW], bf16, tag="esel")
            ident_bf = A.tile([128, 128], bf16, tag="identb")
            wq_sb = W.tile([128, NDC, HQ], bf16, tag="wq")
            wk_sb = W.tile([128, NDC, HD], bf16, tag="wk")
            wv_sb = W.tile([128, NDC, HD], bf16, tag="wv")
            wo_sb = W.tile([128, GQ, D], bf16, tag="wo")
            cos_sb = A.tile([128, T], bf16, tag="cos")
            sin_sb = A.tile([128, T], bf16, tag="sin")
            wqc = A.tile([128, 1], f32, tag="wqc")
            wkc = A.tile([128, 1], f32, tag="wkc")
            wqe = A.tile([128, 1], f32, tag="wqe")
            wke = A.tile([128, 1], f32, tag="wke")

            # issue-order matters: K/V weights + first x quarter first
            xt_q = [None] * NQU
            xt_q0 = BX.tile([128, NDC, QT_W], bf16, tag="xt")
            xt_q[0] = xt_q0
            for c0 in range(0, NDC, 4):
                nc.sync.dma_start(out=wk_sb[:, c0:c0 + 4, :],
                                  in_=wkr[:, c0:c0 + 4, :])
                nc.sync.dma_start(out=xt_q0[:, c0:c0 + 4, :],
                                  in_=xtr[:, c0:c0 + 4, 0:QT_W])
            nc.sync.dma_start(out=wv_sb, in_=wvr)
            nc.sync.dma_start(out=wq_sb[:, 0:8, :], in_=wqr[:, 0:8, :])
            nc.sync.dma_start(out=wq_sb[:, 8:16, :], in_=wqr[:, 8:16, :])
            nc.sync.dma_start(out=cos_sb, in_=cosd[:, :])
            nc.sync.dma_start(out=sin_sb, in_=sind[:, :])
            nc.sync.dma_start(out=wqc, in_=wqcd[:, :])
            nc.sync.dma_start(out=wkc, in_=wkcd[:, :])
            nc.sync.dma_start(out=wqe, in_=wqed[:, :])
            nc.sync.dma_start(out=wke, in_=wked[:, :])
            nc.sync.dma_start(out=onesn_sb, in_=onesnd[:, :])
            xt_q1 = BX.tile([128, NDC, QT_W], bf16, tag="xt")
            xt_q[1] = xt_q1
            nc.sync.dma_start(out=xt_q1, in_=xtr[:, :, QT_W:2 * QT_W])
            nc.sync.dma_start(out=msk_sb, in_=mskr)
            nc.sync.dma_start(out=onesc_sb, in_=onescd[:, :])
            nc.sync.dma_start(out=esel_sb, in_=eseld[:, :])
            nc.sync.dma_start(out=wo_sb, in_=wor)
            make_identity(nc, ident_bf)

            # ---------------- phase 1: projections + norm + rope ----------
            with (
                tc.tile_pool(name="TMP", bufs=2) as TMP,
                tc.tile_pool(name="PKV", bufs=2, space="PSUM") as PKV,
                tc.tile_pool(name="PQ", bufs=2, space="PSUM") as PQ,
                tc.tile_pool(name="PL", bufs=2, space="PSUM") as PL,
                tc.tile_pool(name="PT", bufs=2, space="PSUM") as PT,
            ):
                def normrope(cpsum, wcol, wbias, sl, out_sl):
                    """RMSNorm + norm-weight + RoPE on a [128, 1, 512] PSUM
                    projection; writes bf16 out_sl [128, 1, 512]."""
                    sq = TMP.tile([128, 1, QT_W], f32r, tag="sq")
                    nc.scalar.activation(sq, cpsum, AF.Square)
                    l2 = PL.tile([128, 1, QT_W], f32, tag="l2")
                    nc.tensor.matmul(l2[:, 0, :], onesn_sb, sq[:, 0, :],
                                     start=True, stop=True)
                    sv = TMP.tile([128, 1, QT_W], f32, tag="sv")
                    nc.scalar.activation(sv, l2, AF.Sqrt, scale=wcol, bias=wbias)
                    rc = TMP.tile([128, 1, QT_W], f32, tag="rc")
                    nc.vector.reciprocal(rc, sv)
                    qn = TMP.tile([128, 1, QT_W], bf16, tag="qn")
                    nc.vector.tensor_mul(qn, cpsum, rc)
                    qr = TMP.tile([128, 1, QT_W], bf16, tag="qr")
                    nc.gpsimd.dma_start(out=qr[:64], in_=qn[64:])
                    nc.gpsimd.dma_start(out=qr[64:], in_=qn[:64])
                    t1 = TMP.tile([128, 1, QT_W], bf16, tag="t1")
                    t2 = TMP.tile([128, 1, QT_W], bf16, tag="t2")
                    nc.vector.tensor_mul(t1[:, 0, :], qn[:, 0, :], cos_sb[:, sl])
                    nc.vector.tensor_mul(t2[:, 0, :], qr[:, 0, :], sin_sb[:, sl])
                    nc.vector.tensor_sub(out_sl[:64], t1[:64], t2[:64])
                    nc.vector.tensor_add(out_sl[64:], t1[64:], t2[64:])

                for e in range(NQU):
                    sl = ts(e, QT_W)
                    if 1 <= e < NQU - 1:
                        xt_qn = BX.tile([128, NDC, QT_W], bf16, tag="xt")
                        xt_q[e + 1] = xt_qn
                        nc.sync.dma_start(out=xt_qn,
                                          in_=xtr[:, :, ts(e + 1, QT_W)])
                    xt_t = xt_q[e]
                    # K quarter
                    kp = PKV.tile([128, 1, QT_W], f32, tag="kv")
                    for c in range(NDC):
                        nc.tensor.matmul(kp[:, 0, :], wk_sb[:, c, :], xt_t[:, c, :],
                                         start=(c == 0), stop=(c == NDC - 1))
                    # V quarter
                    vp = PKV.tile([128, 1, QT_W], f32, tag="kv")
                    for c in range(NDC):
                        nc.tensor.matmul(vp[:, 0, :], wv_sb[:, c, :], xt_t[:, c, :],
                                         start=(c == 0), stop=(c == NDC - 1))
                    normrope(kp, wkc, wke, sl, KT[:, :, sl])
                    vt = TMP.tile([128, QT_W], bf16, tag="vt")
                    nc.scalar.activation(vt, vp[:, 0, :], AF.Copy)
                    # Q heads, V transposes interleaved after h0
                    for h in range(GQ):
                        qp = PQ.tile([128, 1, QT_W], f32, tag="qp")
                        for c in range(NDC):
                            nc.tensor.matmul(qp[:, 0, :], wq_sb[:, c, ts(h, 128)],
                                             xt_t[:, c, :],
                                             start=(c == 0), stop=(c == NDC - 1))
                        if h == 1:
                            for i in range(QT_W // 128):
                                tp = PT.tile([128, 128], bf16, tag="tp")
                                nc.tensor.transpose(tp, vt[:, ts(i, 128)], ident_bf)
                                nc.scalar.activation(
                                    Vn[:, e * (QT_W // 128) + i, :], tp, AF.Copy)
                        normrope(qp, wqc, wqe, sl, QT[:, h:h + 1, sl])

            # ---------------- phase 2: attention + out-projection ---------
            with (
                tc.tile_pool(name="CP", bufs=4) as CP,
                tc.tile_pool(name="CT", bufs=4) as CT,
                tc.tile_pool(name="CO", bufs=3) as CO,
                tc.tile_pool(name="SS", bufs=2, space="PSUM") as SS,
                tc.tile_pool(name="OPL", bufs=2, space="PSUM") as OPL,
                tc.tile_pool(name="PB", bufs=2, space="PSUM") as PB,
            ):
                pending = []  # deferred out-proj chunk emitters

                def emit_outproj_chunk(qt, cch):
                    oup = PB.tile([128, OCH], f32, tag="pb")
                    for hc in range(GQ):
                        nc.tensor.matmul(oup, OT[:, hc, ts(qt, 128)],
                                         wo_sb[:, hc, ts(cch, OCH)],
                                         start=(hc == 0), stop=(hc == GQ - 1))
                    ost = CO.tile([128, OCH], f32, tag="ost")
                    nc.scalar.copy(ost, oup)
                    nc.gpsimd.dma_start(
                        out=outd[qt * 128:(qt + 1) * 128, ts(cch, OCH)], in_=ost)

                def emit_lchain(opl, lpacc, h, Jsl):
                    invL = CT.tile([128, 2], bf16, tag="invL")
                    nc.vector.reciprocal(invL, lpacc)
                    invB = PB.tile([128, OCH], f32, tag="pb")
                    # bf16 [2,128] scratch aliased into the unused top half
                    invLT_b = invB[0:2, 256:320].bitcast(bf16)
                    nc.tensor.transpose(invLT_b, invL, ident_bf)
                    invLT_sb = CT.tile([2, 128], bf16, tag="invLTs")
                    nc.vector.tensor_copy(invLT_sb, invLT_b)
                    for qc in range(2):
                        nc.tensor.matmul(invB[:, ts(qc, 128)],
                                         esel_sb[:, ts(qc, 128)],
                                         invLT_sb, start=True, stop=True)
                    invBs = CT.tile([128, JW], bf16, tag="invBs")
                    nc.vector.tensor_copy(invBs, invB[:, 0:JW])
                    nc.vector.tensor_mul(OT[:, h, Jsl], opl[:, 0:JW], invBs)

                deferred = []  # (emit_lchain closure, J, h) queue

                def queue_outproj(J):
                    for qt in range(2 * J, 2 * J + 2):
                        for cch in range(D // OCH):
                            pending.append(
                                lambda qt=qt, cch=cch: emit_outproj_chunk(qt, cch))

                def pop_fill():
                    # one deferred lchain per group slot, then out-proj pops
                    if deferred:
                        fn, dJ, dh = deferred.pop(0)
                        fn()
                        if dh == GQ - 1:
                            queue_outproj(dJ)
                    npop = 2 if len(pending) > 8 else 1
                    for _ in range(npop):
                        if pending:
                            pending.pop(0)()

                def emit_group(st, gi, gw):
                    """one kb-group of chain st = dict(J, Jsl, h, opl, lpacc,
                    kb, nkb, ngroups)"""
                    kb = st["kb"]
                    ss = SS.tile([128, 4, JW], f32, tag="ss")
                    for i in range(gw):
                        nc.tensor.matmul(ss[:, i, :],
                                         KT[:, 0, ts(kb + i, 128)],
                                         QT[:, st["h"], st["Jsl"]],
                                         start=True, stop=True)
                    P = CP.tile([128, 4, JW], bf16, tag="p")
                    nc.scalar.activation(P[:, 0:gw, :], ss[:, 0:gw, :],
                                         AF.Exp, scale=SCALE)
                    if gi == st["ngroups"] - 1:
                        # mask the diagonal pair (last 2 kb blocks)
                        nc.vector.tensor_mul(P[:, gw - 2:gw, :],
                                             P[:, gw - 2:gw, :], msk_sb)
                    # fill the exp-latency window with deferred work
                    pop_fill()
                    # group-local L mini-sums in the score tile's second bank
                    # (closed groups, qc-major), DVE-accumulated into SBUF
                    for qc in range(2):
                        for i in range(gw):
                            nc.tensor.matmul(ss[:, 3, qc:qc + 1],
                                             P[:, i, ts(qc, 128)], onesc_sb,
                                             start=(i == 0), stop=(i == gw - 1))
                    if gi == 0:
                        nc.vector.tensor_copy(st["lpacc"], ss[:, 3, 0:2])
                    else:
                        nc.vector.tensor_add(st["lpacc"], st["lpacc"],
                                             ss[:, 3, 0:2])
                    for i in range(gw):
                        nc.tensor.matmul(st["opl"][:, 0:JW], Vn[:, kb + i, :],
                                         P[:, i, :],
                                         start=(kb + i == 0),
                                         stop=(kb + i == st["nkb"] - 1))
                    st["kb"] = kb + gw

                def new_chain(J, h):
                    nkb = 2 * J + 2
                    opl = OPL.tile([128, OCH], f32, tag="opl")
                    lpacc = CT.tile([128, 2], f32, tag="lpacc")
                    groups = [4] * (nkb // 4) + ([2] if nkb % 4 else [])
                    return {"J": J, "Jsl": ts(J, JW), "h": h, "opl": opl,
                            "lpacc": lpacc, "kb": 0, "nkb": nkb,
                            "ngroups": len(groups), "groups": groups}

                # ascending J, heads interleaved in pairs
                for J in range(NJ):
                    for h0 in (0, 2):
                        ca, cb = new_chain(J, h0), new_chain(J, h0 + 1)
                        for gi in range(ca["ngroups"]):
                            emit_group(ca, gi, ca["groups"][gi])
                            emit_group(cb, gi, cb["groups"][gi])
                        for st in (ca, cb):
                            deferred.append((
                                (lambda st=st: emit_lchain(
                                    st["opl"], st["lpacc"], st["h"],
                                    st["Jsl"])), st["J"], st["h"]))
                while deferred or pending:
                    pop_fill()

    nc.finalize()
    return nc


def _host_consts():
    import ml_dtypes
    bf = ml_dtypes.bfloat16
    inv = 1.0 / (ROPE_BASE ** (np.arange(0, HD, 2, dtype=np.float64) / HD))
    freqs = np.outer(np.arange(T, dtype=np.float64), inv)
    emb = np.concatenate([freqs, freqs], axis=-1)          # [T, HD]
    cosT = np.ascontiguousarray(np.cos(emb).T.astype(np.float32)).astype(bf)
    sinT = np.ascontiguousarray(np.sin(emb).T.astype(np.float32)).astype(bf)
    # diagonal pair mask: msk2[p, i*JW + q] = (128*i + p <= q)
    msk2 = np.zeros((HD, 2 * JW), np.float32)
    for i in range(2):
        k = np.arange(128)[:, None] + 128 * i
        q = np.arange(JW)[None, :]
        msk2[:, i * JW:(i + 1) * JW] = (k <= q).astype(np.float32)
    msk2 = msk2.astype(bf)
    onesn = (np.ones((128, 128), np.float32) / HD)
    onesc = np.ones((128, 1), np.float32).astype(bf)
    esel = np.zeros((2, JW), np.float32)
    for qc in range(2):
        esel[qc, qc * 128:(qc + 1) * 128] = 1.0
    esel = esel.astype(bf)
    return cosT, sinT, msk2, onesn, onesc, esel


def kernel(x, Wq, Wk, Wv, Wo, q_norm_w, k_norm_w):
    import ml_dtypes
    from concourse.bass_utils import run_bass_kernel_spmd
    bf = ml_dtypes.bfloat16

    if "nc" not in _cached:
        _cached["nc"] = _build_program()
        _cached["consts"] = _host_consts()
    nc = _cached["nc"]
    cosT, sinT, msk2, onesn, onesc, esel = _cached["consts"]

    x = np.asarray(x, np.float32)
    Wq = np.asarray(Wq, np.float32)
    Wk = np.asarray(Wk, np.float32)
    Wv = np.asarray(Wv, np.float32)
    Wo = np.asarray(Wo, np.float32)
    qwf = np.asarray(q_norm_w, np.float64).reshape(HD, 1)
    kwf = np.asarray(k_norm_w, np.float64).reshape(HD, 1)
    qw = np.ascontiguousarray((1.0 / qwf ** 2).astype(np.float32))
    kw = np.ascontiguousarray((1.0 / kwf ** 2).astype(np.float32))
    qwe = np.ascontiguousarray((EPS / qwf ** 2).astype(np.float32))
    kwe = np.ascontiguousarray((EPS / kwf ** 2).astype(np.float32))

    xTb = [np.ascontiguousarray(x[b].T).astype(bf) for b in range(B)]
    in_maps = []
    for core in range(8):
        b, kv = divmod(core, NKV)
        in_maps.append({
            "xt": xTb[b],
            "wq": np.ascontiguousarray(Wq[:, kv * HQ:(kv + 1) * HQ]).astype(bf),
            "wk": np.ascontiguousarray(Wk[:, kv * HD:(kv + 1) * HD]).astype(bf),
            "wv": np.ascontiguousarray(Wv[:, kv * HD:(kv + 1) * HD]).astype(bf),
            "wo": np.ascontiguousarray(Wo[kv * HQ:(kv + 1) * HQ, :]).astype(bf),
            "cos": cosT, "sin": sinT,
            "wqc": qw, "wkc": kw, "wqe": qwe, "wke": kwe,
            "msk2": msk2, "onesn": onesn, "onesc": onesc, "esel": esel,
        })
    res = run_bass_kernel_spmd(nc, in_maps, list(range(8)))
    out = np.zeros((B, T, D), np.float64)
    for core in range(8):
        b = core // NKV
        out[b] += res.results[core]["out"].astype(np.float64)
    return out.astype(np.float32)


# revision 30
# speedup vs baseline: 1.3333x; 1.0241x over previous
"""GQA kernel for Trainium2, 8 NeuronCores.

Sharding: core c = b*4 + kv  (b in {0,1} data-parallel over batch,
kv in {0..3} tensor-parallel over the 4 KV head groups; each core owns
4 Q heads + 1 KV head). Each core computes a partial output
x[b] @ Wq[:,kv] -> attention -> @ Wo[kv rows]; host sums the 4 partials
per batch (the row-sharded-Wo all-reduce).

Device layout (per core), bf16 SBUF operands, f32 PSUM accumulation:
  phase 1 (per 512-col quarter of T, per head):
    KT/QT[d,t] = W^T x^T (contraction on partitions, N=512 moving).
    RMSNorm via ones-matmul partition reduction + Act Sqrt with the
    norm weight folded into scale/bias; RoPE via partition-swap DMA
    (SWDGE on the idle gpsimd queue) + DVE bf16 muls.
    Vn (natural [k,d]) via PE transpose.
  phase 2 (per q-slab J of 256, per head, kb groups of <=4 blocks):
    group: S^T = K Q^T (N=256 matmuls into a 2-bank PSUM tile), one
    Act exp over up to [128,1024] -> P bf16, DVE mask-mul on the
    diagonal tail, AV accumulation op += Vn^T P.
    Softmax denominator: near-free N=1 matmuls L[:,qc] += P_chunk^T
    @ ones (P is lhsT; L shares the op PSUM bank), then reciprocal ->
    PE transpose -> selector-matmul broadcast -> one DVE scale mul
    into OT. Out-projection chunks are interleaved between attention
    groups to keep PE saturated; output stores go out over SWDGE.
"""

import numpy as np

B, T, D = 2, 2048, 2048
NH, NKV, HD = 16, 4, 128
GQ = NH // NKV            # 4 q heads per kv head
HQ = GQ * HD              # 512 q-dim per core
ROPE_BASE = 500000.0
EPS = 1e-5
SCALE = 1.0 / np.sqrt(HD)
NQU = 4                   # phase-1 T quarters
QT_W = T // NQU           # 512
NDC = D // 128            # 16 contraction chunks
NJ = 8                    # phase-2 q slabs
JW = T // NJ              # 256
NKB = T // 128            # 16 k blocks
OCH = 512                 # out-projection D chunk

_cached = {}


def _build_program():
    import concourse.bacc as bacc
    import concourse.mybir as mybir
    from concourse import tile
    from concourse.masks import make_identity

    f32 = mybir.dt.float32
    f32r = mybir.dt.float32r
    bf16 = mybir.dt.bfloat16
    AF = mybir.ActivationFunctionType
    from concourse.bass import ts

    nc = bacc.Bacc()

    xt = nc.dram_tensor("xt", [D, T], bf16, kind="ExternalInput")
    wq = nc.dram_tensor("wq", [D, HQ], bf16, kind="ExternalInput")
    wk = nc.dram_tensor("wk", [D, HD], bf16, kind="ExternalInput")
    wv = nc.dram_tensor("wv", [D, HD], bf16, kind="ExternalInput")
    wo = nc.dram_tensor("wo", [HQ, D], bf16, kind="ExternalInput")
    cosd = nc.dram_tensor("cos", [HD, T], bf16, kind="ExternalInput")
    sind = nc.dram_tensor("sin", [HD, T], bf16, kind="ExternalInput")
    wqcd = nc.dram_tensor("wqc", [HD, 1], f32, kind="ExternalInput")
    wkcd = nc.dram_tensor("wkc", [HD, 1], f32, kind="ExternalInput")
    wqed = nc.dram_tensor("wqe", [HD, 1], f32, kind="ExternalInput")
    wked = nc.dram_tensor("wke", [HD, 1], f32, kind="ExternalInput")
    mskd = nc.dram_tensor("msk2", [HD, 2 * JW], bf16, kind="ExternalInput")
    onesnd = nc.dram_tensor("onesn", [128, 128], f32r, kind="ExternalInput")
    onescd = nc.dram_tensor("onesc", [128, 1], bf16, kind="ExternalInput")
    eseld = nc.dram_tensor("esel", [2, JW], bf16, kind="ExternalInput")
    outd = nc.dram_tensor("out", [T, D], f32, kind="ExternalOutput")

    xtr = xt.rearrange("(c p) t -> p c t", p=128)
    wqr = wq.rearrange("(c p) n -> p c n", p=128)
    wkr = wk.rearrange("(c p) n -> p c n", p=128)
    wvr = wv.rearrange("(c p) n -> p c n", p=128)
    wor = wo.rearrange("(c p) n -> p c n", p=128)
    mskr = mskd.rearrange("p (a q) -> p a q", a=2)

    with nc.allow_low_precision(reason="bf16 kernel, tolerance 2e-2"), \
         tile.TileContext(nc) as tc:
        with tc.tile_pool(name="A", bufs=1) as A, \
             tc.tile_pool(name="W", bufs=1) as W, \
             tc.tile_pool(name="BX", bufs=2) as BX:
            # persistent tensors and weights
            QT = A.tile([128, GQ, T], bf16, tag="QT")
            KT = A.tile([128, 1, T], bf16, tag="KT")
            Vn = A.tile([128, NKB, HD], bf16, tag="Vn")
            OT = A.tile([128, GQ, T], bf16, tag="OT")
            msk_sb = A.tile([128, 2, JW], bf16, tag="msk")
            onesn_sb = A.tile([128, 128], f32r, tag="onesn")
            onesc_sb = A.tile([128, 1], bf16, tag="onesc")
            esel_sb = A.tile([2, JW], bf16, tag="esel")
            ident_bf = A.tile([128, 128], bf16, tag="identb")
            wq_sb = W.tile([128, NDC, HQ], bf16, tag="wq")
            wk_sb = W.tile([128, NDC, HD], bf16, tag="wk")
            wv_sb = W.tile([128, NDC, HD], bf16, tag="wv")
            wo_sb = W.tile([128, GQ, D], bf16, tag="wo")
            cos_sb = A.tile([128, T], bf16, tag="cos")
            sin_sb = A.tile([128, T], bf16, tag="sin")
            wqc = A.tile([128, 1], f32, tag="wqc")
            wkc = A.tile([128, 1], f32, tag="wkc")
            wqe = A.tile([128, 1], f32, tag="wqe")
            wke = A.tile([128, 1], f32, tag="wke")

            # issue-order matters: K/V weights + first x quarter first
            xt_q = [None] * NQU
            xt_q0 = BX.tile([128, NDC, QT_W], bf16, tag="xt")
            xt_q[0] = xt_q0
            for c0 in range(0, NDC, 4):
                nc.sync.dma_start(out=wk_sb[:, c0:c0 + 4, :],
                                  in_=wkr[:, c0:c0 + 4, :])
                nc.sync.dma_start(out=xt_q0[:, c0:c0 + 4, :],
                                  in_=xtr[:, c0:c0 + 4, 0:QT_W])
            nc.sync.dma_start(out=wv_sb, in_=wvr)
            nc.sync.dma_start(out=wq_sb[:, 0:8, :], in_=wqr[:, 0:8, :])
            nc.sync.dma_start(out=wq_sb[:, 8:16, :], in_=wqr[:, 8:16, :])
            nc.sync.dma_start(out=cos_sb, in_=cosd[:, :])
            nc.sync.dma_start(out=sin_sb, in_=sind[:, :])
            nc.sync.dma_start(out=wqc, in_=wqcd[:, :])
            nc.sync.dma_start(out=wkc, in_=wkcd[:, :])
            nc.sync.dma_start(out=wqe, in_=wqed[:, :])
            nc.sync.dma_start(out=wke, in_=wked[:, :])
            nc.sync.dma_start(out=onesn_sb, in_=onesnd[:, :])
            xt_q1 = BX.tile([128, NDC, QT_W], bf16, tag="xt")
            xt_q[1] = xt_q1
            nc.sync.dma_start(out=xt_q1, in_=xtr[:, :, QT_W:2 * QT_W])
            nc.sync.dma_start(out=msk_sb, in_=mskr)
            nc.sync.dma_start(out=onesc_sb, in_=onescd[:, :])
            nc.sync.dma_start(out=esel_sb, in_=eseld[:, :])
            nc.sync.dma_start(out=wo_sb, in_=wor)
            make_identity(nc, ident_bf)

            # ---------------- phase 1: projections + norm + rope ----------
            with (
                tc.tile_pool(name="TMP", bufs=2) as TMP,
                tc.tile_pool(name="PKV", bufs=2, space="PSUM") as PKV,
                tc.tile_pool(name="PQ", bufs=3, space="PSUM") as PQ,
                tc.tile_pool(name="PL", bufs=1, space="PSUM") as PL,
                tc.tile_pool(name="PT", bufs=2, space="PSUM") as PT,
            ):
                def normrope(cpsum, wcol, wbias, sl, out_sl):
                    """RMSNorm + norm-weight + RoPE on a [128, 1, 512] PSUM
                    projection; writes bf16 out_sl [128, 1, 512]."""
                    sq = TMP.tile([128, 1, QT_W], f32r, tag="sq")
                    nc.scalar.activation(sq, cpsum, AF.Square)
                    l2 = PL.tile([128, 1, QT_W], f32, tag="l2")
                    nc.tensor.matmul(l2[:, 0, :], onesn_sb, sq[:, 0, :],
                                     start=True, stop=True)
                    sv = TMP.tile([128, 1, QT_W], f32, tag="sv")
                    nc.scalar.activation(sv, l2, AF.Sqrt, scale=wcol, bias=wbias)
                    rc = TMP.tile([128, 1, QT_W], f32, tag="rc")
                    nc.vector.reciprocal(rc, sv)
                    qn = TMP.tile([128, 1, QT_W], bf16, tag="qn")
                    nc.vector.tensor_mul(qn, cpsum, rc)
                    qr = TMP.tile([128, 1, QT_W], bf16, tag="qr")
                    nc.gpsimd.dma_start(out=qr[:64], in_=qn[64:])
                    nc.gpsimd.dma_start(out=qr[64:], in_=qn[:64])
                    t1 = TMP.tile([128, 1, QT_W], bf16, tag="t1")
                    t2 = TMP.tile([128, 1, QT_W], bf16, tag="t2")
                    nc.vector.tensor_mul(t1[:, 0, :], qn[:, 0, :], cos_sb[:, sl])
                    nc.vector.tensor_mul(t2[:, 0, :], qr[:, 0, :], sin_sb[:, sl])
                    nc.vector.tensor_sub(out_sl[:64], t1[:64], t2[:64])
                    nc.vector.tensor_add(out_sl[64:], t1[64:], t2[64:])

                for e in range(NQU):
                    sl = ts(e, QT_W)
                    if 1 <= e < NQU - 1:
                        xt_qn = BX.tile([128, NDC, QT_W], bf16, tag="xt")
                        xt_q[e + 1] = xt_qn
                        nc.sync.dma_start(out=xt_qn,
                                          in_=xtr[:, :, ts(e + 1, QT_W)])
                    xt_t = xt_q[e]
                    # K quarter
                    kp = PKV.tile([128, 1, QT_W], f32, tag="kv")
                    for c in range(NDC):
                        nc.tensor.matmul(kp[:, 0, :], wk_sb[:, c, :], xt_t[:, c, :],
                                         start=(c == 0), stop=(c == NDC - 1))
                    # V quarter
                    vp = PKV.tile([128, 1, QT_W], f32, tag="kv")
                    for c in range(NDC):
                        nc.tensor.matmul(vp[:, 0, :], wv_sb[:, c, :], xt_t[:, c, :],
                                         start=(c == 0), stop=(c == NDC - 1))
                    normrope(kp, wkc, wke, sl, KT[:, :, sl])
                    vt = TMP.tile([128, QT_W], bf16, tag="vt")
                    nc.scalar.activation(vt, vp[:, 0, :], AF.Copy)
                    # Q heads, V transposes interleaved after h0
                    for h in range(GQ):
                        qp = PQ.tile([128, 1, QT_W], f32, tag="qp")
                        for c in range(NDC):
                            nc.tensor.matmul(qp[:, 0, :], wq_sb[:, c, ts(h, 128)],
                                             xt_t[:, c, :],
                                             start=(c == 0), stop=(c == NDC - 1))
                        if h == 1:
                            for i in range(QT_W // 128):
                                tp = PT.tile([128, 128], bf16, tag="tp")
                                nc.tensor.transpose(tp, vt[:, ts(i, 128)], ident_bf)
                                nc.scalar.activation(
                                    Vn[:, e * (QT_W // 128) + i, :], tp, AF.Copy)
                        normrope(qp, wqc, wqe, sl, QT[:, h:h + 1, sl])

            # ---------------- phase 2: attention + out-projection ---------
            with (
                tc.tile_pool(name="CP", bufs=6) as CP,
                tc.tile_pool(name="CT", bufs=4) as CT,
                tc.tile_pool(name="CO", bufs=3) as CO,
                tc.tile_pool(name="SS", bufs=2, space="PSUM") as SS,
                tc.tile_pool(name="OPL", bufs=2, space="PSUM") as OPL,
                tc.tile_pool(name="PB", bufs=2, space="PSUM") as PB,
            ):
                pending = []  # deferred out-proj chunk emitters

                def emit_outproj_chunk(qt, cch):
                    oup = PB.tile([128, OCH], f32, tag="pb")
                    for hc in range(GQ):
                        nc.tensor.matmul(oup, OT[:, hc, ts(qt, 128)],
                                         wo_sb[:, hc, ts(cch, OCH)],
                                         start=(hc == 0), stop=(hc == GQ - 1))
                    ost = CO.tile([128, OCH], f32, tag="ost")
                    nc.scalar.copy(ost, oup)
                    nc.gpsimd.dma_start(
                        out=outd[qt * 128:(qt + 1) * 128, ts(cch, OCH)], in_=ost)

                def emit_lchain(opl, lpacc, h, Jsl):
                    invL = CT.tile([128, 2], bf16, tag="invL")
                    nc.vector.reciprocal(invL, lpacc)
                    invB = PB.tile([128, OCH], f32, tag="pb")
                    # bf16 [2,128] scratch aliased into the unused top half
                    invLT_b = invB[0:2, 256:320].bitcast(bf16)
                    nc.tensor.transpose(invLT_b, invL, ident_bf)
                    invLT_sb = CT.tile([2, 128], bf16, tag="invLTs")
                    nc.vector.tensor_copy(invLT_sb, invLT_b)
                    for qc in range(2):
                        nc.tensor.matmul(invB[:, ts(qc, 128)],
                                         esel_sb[:, ts(qc, 128)],
                                         invLT_sb, start=True, stop=True)
                    invBs = CT.tile([128, JW], bf16, tag="invBs")
                    nc.vector.tensor_copy(invBs, invB[:, 0:JW])
                    nc.vector.tensor_mul(OT[:, h, Jsl], opl[:, 0:JW], invBs)

                deferred = []  # (emit_lchain closure, J, h) queue

                def queue_outproj(J):
                    for qt in range(2 * J, 2 * J + 2):
                        for cch in range(D // OCH):
                            pending.append(
                                lambda qt=qt, cch=cch: emit_outproj_chunk(qt, cch))

                def pop_fill():
                    # one deferred lchain per group slot, then out-proj pops
                    if deferred:
                        fn, dJ, dh = deferred.pop(0)
                        fn()
                        if dh == GQ - 1:
                            queue_outproj(dJ)
                    if pending:
                        pending.pop(0)()

                def emit_group(st, gi, gw):
                    """one kb-group of chain st = dict(J, Jsl, h, opl, lpacc,
                    kb, nkb, ngroups)"""
                    kb = st["kb"]
                    ss = SS.tile([128, 4, JW], f32, tag="ss")
                    for i in range(gw):
                        nc.tensor.matmul(ss[:, i, :],
                                         KT[:, 0, ts(kb + i, 128)],
                                         QT[:, st["h"], st["Jsl"]],
                                         start=True, stop=True)
                    P = CP.tile([128, 4, JW], bf16, tag="p")
                    nc.scalar.activation(P[:, 0:gw, :], ss[:, 0:gw, :],
                                         AF.Exp, scale=SCALE)
                    if gi == st["ngroups"] - 1:
                        # mask the diagonal pair (last 2 kb blocks)
                        nc.vector.tensor_mul(P[:, gw - 2:gw, :],
                                             P[:, gw - 2:gw, :], msk_sb)
                    # fill the exp-latency window with deferred work
                    pop_fill()
                    # group-local L mini-sums in the score tile's second bank
                    # (closed groups, qc-major), DVE-accumulated into SBUF
                    for qc in range(2):
                        for i in range(gw):
                            nc.tensor.matmul(ss[:, 3, qc:qc + 1],
                                             P[:, i, ts(qc, 128)], onesc_sb,
                                             start=(i == 0), stop=(i == gw - 1))
                    if gi == 0:
                        nc.vector.tensor_copy(st["lpacc"], ss[:, 3, 0:2])
                    else:
                        nc.vector.tensor_add(st["lpacc"], st["lpacc"],
                                             ss[:, 3, 0:2])
                    for i in range(gw):
                        nc.tensor.matmul(st["opl"][:, 0:JW], Vn[:, kb + i, :],
                                         P[:, i, :],
                                         start=(kb + i == 0),
                                         stop=(kb + i == st["nkb"] - 1))
                    st["kb"] = kb + gw

                def new_chain(J, h):
                    nkb = 2 * J + 2
                    opl = OPL.tile([128, OCH], f32, tag="opl")
                    lpacc = CT.tile([128, 2], f32, tag="lpacc")
                    groups = [4] * (nkb // 4) + ([2] if nkb % 4 else [])
                    return {"J": J, "Jsl": ts(J, JW), "h": h, "opl": opl,
                            "lpacc": lpacc, "kb": 0, "nkb": nkb,
                            "ngroups": len(groups), "groups": groups}

                # ascending J, heads interleaved in pairs
                for J in range(NJ):
                    for h0 in (0, 2):
                        ca, cb = new_chain(J, h0), new_chain(J, h0 + 1)
                        for gi in range(ca["ngroups"]):
                            emit_group(ca, gi, ca["groups"][gi])
                            emit_group(cb, gi, cb["groups"][gi])
                        for st in (ca, cb):
                            deferred.append((
                                (lambda st=st: emit_lchain(
                                    st["opl"], st["lpacc"], st["h"],
                                    st["Jsl"])), st["J"], st["h"]))
                while deferred or pending:
                    pop_fill()

    nc.finalize()
    return nc


def _host_consts():
    import ml_dtypes
    bf = ml_dtypes.bfloat16
    inv = 1.0 / (ROPE_BASE ** (np.arange(0, HD, 2, dtype=np.float64) / HD))
    freqs = np.outer(np.arange(T, dtype=np.float64), inv)
    emb = np.concatenate([freqs, freqs], axis=-1)          # [T, HD]
    cosT = np.ascontiguousarray(np.cos(emb).T.astype(np.float32)).astype(bf)
    sinT = np.ascontiguousarray(np.sin(emb).T.astype(np.float32)).astype(bf)
    # diagonal pair mask: msk2[p, i*JW + q] = (128*i + p <= q)
    msk2 = np.zeros((HD, 2 * JW), np.float32)
    for i in range(2):
        k = np.arange(128)[:, None] + 128 * i
        q = np.arange(JW)[None, :]
        msk2[:, i * JW:(i + 1) * JW] = (k <= q).astype(np.float32)
    msk2 = msk2.astype(bf)
    onesn = (np.ones((128, 128), np.float32) / HD)
    onesc = np.ones((128, 1), np.float32).astype(bf)
    esel = np.zeros((2, JW), np.float32)
    for qc in range(2):
        esel[qc, qc * 128:(qc + 1) * 128] = 1.0
    esel = esel.astype(bf)
    return cosT, sinT, msk2, onesn, onesc, esel


def kernel(x, Wq, Wk, Wv, Wo, q_norm_w, k_norm_w):
    import ml_dtypes
    from concourse.bass_utils import run_bass_kernel_spmd
    bf = ml_dtypes.bfloat16

    if "nc" not in _cached:
        _cached["nc"] = _build_program()
        _cached["consts"] = _host_consts()
    nc = _cached["nc"]
    cosT, sinT, msk2, onesn, onesc, esel = _cached["consts"]

    x = np.asarray(x, np.float32)
    Wq = np.asarray(Wq, np.float32)
    Wk = np.asarray(Wk, np.float32)
    Wv = np.asarray(Wv, np.float32)
    Wo = np.asarray(Wo, np.float32)
    qwf = np.asarray(q_norm_w, np.float64).reshape(HD, 1)
    kwf = np.asarray(k_norm_w, np.float64).reshape(HD, 1)
    qw = np.ascontiguousarray((1.0 / qwf ** 2).astype(np.float32))
    kw = np.ascontiguousarray((1.0 / kwf ** 2).astype(np.float32))
    qwe = np.ascontiguousarray((EPS / qwf ** 2).astype(np.float32))
    kwe = np.ascontiguousarray((EPS / kwf ** 2).astype(np.float32))

    xTb = [np.ascontiguousarray(x[b].T).astype(bf) for b in range(B)]
    in_maps = []
    for core in range(8):
        b, kv = divmod(core, NKV)
        in_maps.append({
            "xt": xTb[b],
            "wq": np.ascontiguousarray(Wq[:, kv * HQ:(kv + 1) * HQ]).astype(bf),
            "wk": np.ascontiguousarray(Wk[:, kv * HD:(kv + 1) * HD]).astype(bf),
            "wv": np.ascontiguousarray(Wv[:, kv * HD:(kv + 1) * HD]).astype(bf),
            "wo": np.ascontiguousarray(Wo[kv * HQ:(kv + 1) * HQ, :]).astype(bf),
            "cos": cosT, "sin": sinT,
            "wqc": qw, "wkc": kw, "wqe": qwe, "wke": kwe,
            "msk2": msk2, "onesn": onesn, "onesc": onesc, "esel": esel,
        })
    res = run_bass_kernel_spmd(nc, in_maps, list(range(8)))
    out = np.zeros((B, T, D), np.float64)
    for core in range(8):
        b = core // NKV
        out[b] += res.results[core]["out"].astype(np.float64)
    return out.astype(np.float32)


# revision 33
# speedup vs baseline: 1.3519x; 1.0139x over previous
"""GQA kernel for Trainium2, 8 NeuronCores.

Sharding: core c = b*4 + kv  (b in {0,1} data-parallel over batch,
kv in {0..3} tensor-parallel over the 4 KV head groups; each core owns
4 Q heads + 1 KV head). Each core computes a partial output
x[b] @ Wq[:,kv] -> attention -> @ Wo[kv rows]; host sums the 4 partials
per batch (the row-sharded-Wo all-reduce).

Device layout (per core), bf16 SBUF operands, f32 PSUM accumulation:
  phase 1 (per 512-col quarter of T, per head):
    KT/QT[d,t] = W^T x^T (contraction on partitions, N=512 moving).
    RMSNorm via ones-matmul partition reduction + Act Sqrt with the
    norm weight folded into scale/bias; RoPE via partition-swap DMA
    (SWDGE on the idle gpsimd queue) + DVE bf16 muls.
    Vn (natural [k,d]) via PE transpose.
  phase 2 (per q-slab J of 256, per head, kb groups of <=4 blocks):
    group: S^T = K Q^T (N=256 matmuls into a 2-bank PSUM tile), one
    Act exp over up to [128,1024] -> P bf16, DVE mask-mul on the
    diagonal tail, AV accumulation op += Vn^T P.
    Softmax denominator: near-free N=1 matmuls L[:,qc] += P_chunk^T
    @ ones (P is lhsT; L shares the op PSUM bank), then reciprocal ->
    PE transpose -> selector-matmul broadcast -> one DVE scale mul
    into OT. Out-projection chunks are interleaved between attention
    groups to keep PE saturated; output stores go out on the SP queue.
"""

import numpy as np

B, T, D = 2, 2048, 2048
NH, NKV, HD = 16, 4, 128
GQ = NH // NKV            # 4 q heads per kv head
HQ = GQ * HD              # 512 q-dim per core
ROPE_BASE = 500000.0
EPS = 1e-5
SCALE = 1.0 / np.sqrt(HD)
NQU = 4                   # phase-1 T quarters
QT_W = T // NQU           # 512
NDC = D // 128            # 16 contraction chunks
NJ = 8                    # phase-2 q slabs
JW = T // NJ              # 256
NKB = T // 128            # 16 k blocks
OCH = 512                 # out-projection D chunk

_cached = {}


def _build_program():
    import concourse.bacc as bacc
    import concourse.mybir as mybir
    from concourse import tile
    from concourse.masks import make_identity

    f32 = mybir.dt.float32
    f32r = mybir.dt.float32r
    bf16 = mybir.dt.bfloat16
    AF = mybir.ActivationFunctionType
    from concourse.bass import ts

    nc = bacc.Bacc()

    xt = nc.dram_tensor("xt", [D, T], bf16, kind="ExternalInput")
    wq = nc.dram_tensor("wq", [D, HQ], bf16, kind="ExternalInput")
    wk = nc.dram_tensor("wk", [D, HD], bf16, kind="ExternalInput")
    wv = nc.dram_tensor("wv", [D, HD], bf16, kind="ExternalInput")
    wo = nc.dram_tensor("wo", [HQ, D], bf16, kind="ExternalInput")
    cosd = nc.dram_tensor("cos", [HD, T], bf16, kind="ExternalInput")
    sind = nc.dram_tensor("sin", [HD, T], bf16, kind="ExternalInput")
    wqcd = nc.dram_tensor("wqc", [HD, 1], f32, kind="ExternalInput")
    wkcd = nc.dram_tensor("wkc", [HD, 1], f32, kind="ExternalInput")
    wqed = nc.dram_tensor("wqe", [HD, 1], f32, kind="ExternalInput")
    wked = nc.dram_tensor("wke", [HD, 1], f32, kind="ExternalInput")
    mskd = nc.dram_tensor("msk2", [HD, 2 * JW], bf16, kind="ExternalInput")
    onesnd = nc.dram_tensor("onesn", [128, 128], f32r, kind="ExternalInput")
    onescd = nc.dram_tensor("onesc", [128, 1], bf16, kind="ExternalInput")
    eseld = nc.dram_tensor("esel", [2, JW], bf16, kind="ExternalInput")
    outd = nc.dram_tensor("out", [T, D], f32, kind="ExternalOutput")

    xtr = xt.rearrange("(c p) t -> p c t", p=128)
    wqr = wq.rearrange("(c p) n -> p c n", p=128)
    wkr = wk.rearrange("(c p) n -> p c n", p=128)
    wvr = wv.rearrange("(c p) n -> p c n", p=128)
    wor = wo.rearrange("(c p) n -> p c n", p=128)
    mskr = mskd.rearrange("p (a q) -> p a q", a=2)

    with nc.allow_low_precision(reason="bf16 kernel, tolerance 2e-2"), \
         tile.TileContext(nc) as tc:
        with tc.tile_pool(name="A", bufs=1) as A, \
             tc.tile_pool(name="W", bufs=1) as W, \
             tc.tile_pool(name="BX", bufs=2) as BX:
            # persistent tensors and weights
            QT = A.tile([128, GQ, T], bf16, tag="QT")
            KT = A.tile([128, 1, T], bf16, tag="KT")
            Vn = A.tile([128, NKB, HD], bf16, tag="Vn")
            OT = A.tile([128, GQ, T], bf16, tag="OT")
            msk_sb = A.tile([128, 2, JW], bf16, tag="msk")
            onesn_sb = A.tile([128, 128], f32r, tag="onesn")
            onesc_sb = A.tile([128, 1], bf16, tag="onesc")
            esel_sb = A.tile([2, JW], bf16, tag="esel")
            ident_bf = A.tile([128, 128], bf16, tag="identb")
            wq_sb = W.tile([128, NDC, HQ], bf16, tag="wq")
            wk_sb = W.tile([128, NDC, HD], bf16, tag="wk")
            wv_sb = W.tile([128, NDC, HD], bf16, tag="wv")
            wo_sb = W.tile([128, GQ, D], bf16, tag="wo")
            cos_sb = A.tile([128, T], bf16, tag="cos")
            sin_sb = A.tile([128, T], bf16, tag="sin")
            wqc = A.tile([128, 1], f32, tag="wqc")
            wkc = A.tile([128, 1], f32, tag="wkc")
            wqe = A.tile([128, 1], f32, tag="wqe")
            wke = A.tile([128, 1], f32, tag="wke")

            # issue-order matters: K/V weights + first x quarter first
            xt_q = [None] * NQU
            xt_q0 = BX.tile([128, NDC, QT_W], bf16, tag="xt")
            xt_q[0] = xt_q0
            for c0 in range(0, NDC, 4):
                nc.sync.dma_start(out=wk_sb[:, c0:c0 + 4, :],
                                  in_=wkr[:, c0:c0 + 4, :])
                nc.sync.dma_start(out=xt_q0[:, c0:c0 + 4, :],
                                  in_=xtr[:, c0:c0 + 4, 0:QT_W])
            nc.sync.dma_start(out=wv_sb, in_=wvr)
            nc.sync.dma_start(out=wq_sb[:, 0:8, :], in_=wqr[:, 0:8, :])
            nc.sync.dma_start(out=wq_sb[:, 8:16, :], in_=wqr[:, 8:16, :])
            nc.sync.dma_start(out=cos_sb, in_=cosd[:, :])
            nc.sync.dma_start(out=sin_sb, in_=sind[:, :])
            nc.sync.dma_start(out=wqc, in_=wqcd[:, :])
            nc.sync.dma_start(out=wkc, in_=wkcd[:, :])
            nc.sync.dma_start(out=wqe, in_=wqed[:, :])
            nc.sync.dma_start(out=wke, in_=wked[:, :])
            nc.sync.dma_start(out=onesn_sb, in_=onesnd[:, :])
            xt_q1 = BX.tile([128, NDC, QT_W], bf16, tag="xt")
            xt_q[1] = xt_q1
            nc.sync.dma_start(out=xt_q1, in_=xtr[:, :, QT_W:2 * QT_W])
            nc.sync.dma_start(out=msk_sb, in_=mskr)
            nc.sync.dma_start(out=onesc_sb, in_=onescd[:, :])
            nc.sync.dma_start(out=esel_sb, in_=eseld[:, :])
            nc.sync.dma_start(out=wo_sb, in_=wor)
            make_identity(nc, ident_bf)

            # ---------------- phase 1: projections + norm + rope ----------
            with (
                tc.tile_pool(name="TMP", bufs=2) as TMP,
                tc.tile_pool(name="PKV", bufs=2, space="PSUM") as PKV,
                tc.tile_pool(name="PQ", bufs=3, space="PSUM") as PQ,
                tc.tile_pool(name="PL", bufs=1, space="PSUM") as PL,
                tc.tile_pool(name="PT", bufs=2, space="PSUM") as PT,
            ):
                def normrope(cpsum, wcol, wbias, sl, out_sl):
                    """RMSNorm + norm-weight + RoPE on a [128, 1, 512] PSUM
                    projection; writes bf16 out_sl [128, 1, 512]."""
                    sq = TMP.tile([128, 1, QT_W], f32r, tag="sq")
                    nc.scalar.activation(sq, cpsum, AF.Square)
                    l2 = PL.tile([128, 1, QT_W], f32, tag="l2")
                    nc.tensor.matmul(l2[:, 0, :], onesn_sb, sq[:, 0, :],
                                     start=True, stop=True)
                    sv = TMP.tile([128, 1, QT_W], f32, tag="sv")
                    nc.scalar.activation(sv, l2, AF.Sqrt, scale=wcol, bias=wbias)
                    rc = TMP.tile([128, 1, QT_W], f32, tag="rc")
                    nc.vector.reciprocal(rc, sv)
                    qn = TMP.tile([128, 1, QT_W], bf16, tag="qn")
                    nc.vector.tensor_mul(qn, cpsum, rc)
                    qr = TMP.tile([128, 1, QT_W], bf16, tag="qr")
                    nc.gpsimd.dma_start(out=qr[:64], in_=qn[64:])
                    nc.gpsimd.dma_start(out=qr[64:], in_=qn[:64])
                    t1 = TMP.tile([128, 1, QT_W], bf16, tag="t1")
                    t2 = TMP.tile([128, 1, QT_W], bf16, tag="t2")
                    nc.vector.tensor_mul(t1[:, 0, :], qn[:, 0, :], cos_sb[:, sl])
                    nc.vector.tensor_mul(t2[:, 0, :], qr[:, 0, :], sin_sb[:, sl])
                    nc.vector.tensor_sub(out_sl[:64], t1[:64], t2[:64])
                    nc.vector.tensor_add(out_sl[64:], t1[64:], t2[64:])

                for e in range(NQU):
                    sl = ts(e, QT_W)
                    if 1 <= e < NQU - 1:
                        xt_qn = BX.tile([128, NDC, QT_W], bf16, tag="xt")
                        xt_q[e + 1] = xt_qn
                        nc.sync.dma_start(out=xt_qn,
                                          in_=xtr[:, :, ts(e + 1, QT_W)])
                    xt_t = xt_q[e]
                    # K quarter
                    kp = PKV.tile([128, 1, QT_W], f32, tag="kv")
                    for c in range(NDC):
                        nc.tensor.matmul(kp[:, 0, :], wk_sb[:, c, :], xt_t[:, c, :],
                                         start=(c == 0), stop=(c == NDC - 1))
                    # V quarter
                    vp = PKV.tile([128, 1, QT_W], f32, tag="kv")
                    for c in range(NDC):
                        nc.tensor.matmul(vp[:, 0, :], wv_sb[:, c, :], xt_t[:, c, :],
                                         start=(c == 0), stop=(c == NDC - 1))
                    normrope(kp, wkc, wke, sl, KT[:, :, sl])
                    vt = TMP.tile([128, QT_W], bf16, tag="vt")
                    nc.scalar.activation(vt, vp[:, 0, :], AF.Copy)
                    # Q heads, V transposes interleaved after h0
                    for h in range(GQ):
                        qp = PQ.tile([128, 1, QT_W], f32, tag="qp")
                        for c in range(NDC):
                            nc.tensor.matmul(qp[:, 0, :], wq_sb[:, c, ts(h, 128)],
                                             xt_t[:, c, :],
                                             start=(c == 0), stop=(c == NDC - 1))
                        if h == 1:
                            for i in range(QT_W // 128):
                                tp = PT.tile([128, 128], bf16, tag="tp")
                                nc.tensor.transpose(tp, vt[:, ts(i, 128)], ident_bf)
                                nc.scalar.activation(
                                    Vn[:, e * (QT_W // 128) + i, :], tp, AF.Copy)
                        normrope(qp, wqc, wqe, sl, QT[:, h:h + 1, sl])

            # ---------------- phase 2: attention + out-projection ---------
            with (
                tc.tile_pool(name="CP", bufs=6) as CP,
                tc.tile_pool(name="CT", bufs=6) as CT,
                tc.tile_pool(name="CO", bufs=4) as CO,
                tc.tile_pool(name="SS", bufs=2, space="PSUM") as SS,
                tc.tile_pool(name="OPL", bufs=2, space="PSUM") as OPL,
                tc.tile_pool(name="PB", bufs=2, space="PSUM") as PB,
            ):
                pending = []  # deferred out-proj chunk emitters

                def emit_outproj_chunk(qt, cch):
                    oup = PB.tile([128, OCH], f32, tag="pb")
                    for hc in range(GQ):
                        nc.tensor.matmul(oup, OT[:, hc, ts(qt, 128)],
                                         wo_sb[:, hc, ts(cch, OCH)],
                                         start=(hc == 0), stop=(hc == GQ - 1))
                    ost = CO.tile([128, OCH], f32, tag="ost")
                    nc.scalar.copy(ost, oup)
                    nc.sync.dma_start(
                        out=outd[qt * 128:(qt + 1) * 128, ts(cch, OCH)], in_=ost)

                def emit_lchain(opl, lpacc, h, Jsl):
                    invL = CT.tile([128, 2], bf16, tag="invL")
                    nc.vector.reciprocal(invL, lpacc)
                    invB = PB.tile([128, OCH], f32, tag="pb")
                    # bf16 [2,128] scratch aliased into the unused top half
                    invLT_b = invB[0:2, 256:320].bitcast(bf16)
                    nc.tensor.transpose(invLT_b, invL, ident_bf)
                    invLT_sb = CT.tile([2, 128], bf16, tag="invLTs")
                    nc.vector.tensor_copy(invLT_sb, invLT_b)
                    for qc in range(2):
                        nc.tensor.matmul(invB[:, ts(qc, 128)],
                                         esel_sb[:, ts(qc, 128)],
                                         invLT_sb, start=True, stop=True)
                    invBs = CT.tile([128, JW], bf16, tag="invBs")
                    nc.vector.tensor_copy(invBs, invB[:, 0:JW])
                    nc.vector.tensor_mul(OT[:, h, Jsl], opl[:, 0:JW], invBs)

                deferred = []  # (emit_lchain closure, J, h) queue

                def queue_outproj(J):
                    for qt in range(2 * J, 2 * J + 2):
                        for cch in range(D // OCH):
                            pending.append(
                                lambda qt=qt, cch=cch: emit_outproj_chunk(qt, cch))

                def pop_fill():
                    # one deferred lchain per group slot, then out-proj pops
                    if deferred:
                        fn, dJ, dh = deferred.pop(0)
                        fn()
                        if dh == GQ - 1:
                            queue_outproj(dJ)
                    if pending:
                        pending.pop(0)()

                def emit_group(st, gi, gw):
                    """one kb-group of chain st = dict(J, Jsl, h, opl, lpacc,
                    kb, nkb, ngroups)"""
                    kb = st["kb"]
                    ss = SS.tile([128, 4, JW], f32, tag="ss")
                    for i in range(gw):
                        nc.tensor.matmul(ss[:, i, :],
                                         KT[:, 0, ts(kb + i, 128)],
                                         QT[:, st["h"], st["Jsl"]],
                                         start=True, stop=True)
                    P = CP.tile([128, 4, JW], bf16, tag="p")
                    nc.scalar.activation(P[:, 0:gw, :], ss[:, 0:gw, :],
                                         AF.Exp, scale=SCALE)
                    if gi == st["ngroups"] - 1:
                        # mask the diagonal pair (last 2 kb blocks)
                        nc.vector.tensor_mul(P[:, gw - 2:gw, :],
                                             P[:, gw - 2:gw, :], msk_sb)
                    # fill the exp-latency window with deferred work
                    pop_fill()
                    # group-local L mini-sums in the score tile's second bank
                    # (closed groups, qc-major), DVE-accumulated into SBUF
                    for qc in range(2):
                        for i in range(gw):
                            nc.tensor.matmul(ss[:, 3, qc:qc + 1],
                                             P[:, i, ts(qc, 128)], onesc_sb,
                                             start=(i == 0), stop=(i == gw - 1))
                    if gi == 0:
                        nc.vector.tensor_copy(st["lpacc"], ss[:, 3, 0:2])
                    else:
                        nc.vector.tensor_add(st["lpacc"], st["lpacc"],
                                             ss[:, 3, 0:2])
                    for i in range(gw):
                        nc.tensor.matmul(st["opl"][:, 0:JW], Vn[:, kb + i, :],
                                         P[:, i, :],
                                         start=(kb + i == 0),
                                         stop=(kb + i == st["nkb"] - 1))
                    st["kb"] = kb + gw

                def new_chain(J, h):
                    nkb = 2 * J + 2
                    opl = OPL.tile([128, OCH], f32, tag="opl")
                    lpacc = CT.tile([128, 2], f32, tag="lpacc")
                    groups = [4] * (nkb // 4) + ([2] if nkb % 4 else [])
                    return {"J": J, "Jsl": ts(J, JW), "h": h, "opl": opl,
                            "lpacc": lpacc, "kb": 0, "nkb": nkb,
                            "ngroups": len(groups), "groups": groups}

                # ascending J, heads interleaved in pairs
                for J in range(NJ):
                    for h0 in (0, 2):
                        ca, cb = new_chain(J, h0), new_chain(J, h0 + 1)
                        for gi in range(ca["ngroups"]):
                            emit_group(ca, gi, ca["groups"][gi])
                            emit_group(cb, gi, cb["groups"][gi])
                        for st in (ca, cb):
                            deferred.append((
                                (lambda st=st: emit_lchain(
                                    st["opl"], st["lpacc"], st["h"],
                                    st["Jsl"])), st["J"], st["h"]))
                while deferred or pending:
                    pop_fill()

    nc.finalize()
    return nc


def _host_consts():
    import ml_dtypes
    bf = ml_dtypes.bfloat16
    inv = 1.0 / (ROPE_BASE ** (np.arange(0, HD, 2, dtype=np.float64) / HD))
    freqs = np.outer(np.arange(T, dtype=np.float64), inv)
    emb = np.concatenate([freqs, freqs], axis=-1)          # [T, HD]
    cosT = np.ascontiguousarray(np.cos(emb).T.astype(np.float32)).astype(bf)
    sinT = np.ascontiguousarray(np.sin(emb).T.astype(np.float32)).astype(bf)
    # diagonal pair mask: msk2[p, i*JW + q] = (128*i + p <= q)
    msk2 = np.zeros((HD, 2 * JW), np.float32)
    for i in range(2):
        k = np.arange(128)[:, None] + 128 * i
        q = np.arange(JW)[None, :]
        msk2[:, i * JW:(i + 1) * JW] = (k <= q).astype(np.float32)
    msk2 = msk2.astype(bf)
    onesn = (np.ones((128, 128), np.float32) / HD)
    onesc = np.ones((128, 1), np.float32).astype(bf)
    esel = np.zeros((2, JW), np.float32)
    for qc in range(2):
        esel[qc, qc * 128:(qc + 1) * 128] = 1.0
    esel = esel.astype(bf)
    return cosT, sinT, msk2, onesn, onesc, esel


def kernel(x, Wq, Wk, Wv, Wo, q_norm_w, k_norm_w):
    import ml_dtypes
    from concourse.bass_utils import run_bass_kernel_spmd
    bf = ml_dtypes.bfloat16

    if "nc" not in _cached:
        _cached["nc"] = _build_program()
        _cached["consts"] = _host_consts()
    nc = _cached["nc"]
    cosT, sinT, msk2, onesn, onesc, esel = _cached["consts"]

    x = np.asarray(x, np.float32)
    Wq = np.asarray(Wq, np.float32)
    Wk = np.asarray(Wk, np.float32)
    Wv = np.asarray(Wv, np.float32)
    Wo = np.asarray(Wo, np.float32)
    qwf = np.asarray(q_norm_w, np.float64).reshape(HD, 1)
    kwf = np.asarray(k_norm_w, np.float64).reshape(HD, 1)
    qw = np.ascontiguousarray((1.0 / qwf ** 2).astype(np.float32))
    kw = np.ascontiguousarray((1.0 / kwf ** 2).astype(np.float32))
    qwe = np.ascontiguousarray((EPS / qwf ** 2).astype(np.float32))
    kwe = np.ascontiguousarray((EPS / kwf ** 2).astype(np.float32))

    xTb = [np.ascontiguousarray(x[b].T).astype(bf) for b in range(B)]
    in_maps = []
    for core in range(8):
        b, kv = divmod(core, NKV)
        in_maps.append({
            "xt": xTb[b],
            "wq": np.ascontiguousarray(Wq[:, kv * HQ:(kv + 1) * HQ]).astype(bf),
            "wk": np.ascontiguousarray(Wk[:, kv * HD:(kv + 1) * HD]).astype(bf),
            "wv": np.ascontiguousarray(Wv[:, kv * HD:(kv + 1) * HD]).astype(bf),
            "wo": np.ascontiguousarray(Wo[kv * HQ:(kv + 1) * HQ, :]).astype(bf),
            "cos": cosT, "sin": sinT,
            "wqc": qw, "wkc": kw, "wqe": qwe, "wke": kwe,
            "msk2": msk2, "onesn": onesn, "onesc": onesc, "esel": esel,
        })
    res = run_bass_kernel_spmd(nc, in_maps, list(range(8)))
    out = np.zeros((B, T, D), np.float64)
    for core in range(8):
        b = core // NKV
        out[b] += res.results[core]["out"].astype(np.float64)
    return out.astype(np.float32)


# revision 34
# speedup vs baseline: 1.3982x; 1.0342x over previous
"""GQA kernel for Trainium2, 8 NeuronCores.

Sharding: core c = b*4 + kv  (b in {0,1} data-parallel over batch,
kv in {0..3} tensor-parallel over the 4 KV head groups; each core owns
4 Q heads + 1 KV head). Each core computes a partial output
x[b] @ Wq[:,kv] -> attention -> @ Wo[kv rows]; host sums the 4 partials
per batch (the row-sharded-Wo all-reduce).

Device layout (per core), bf16 SBUF operands, f32 PSUM accumulation:
  phase 1 (per 512-col quarter of T, per head):
    KT/QT[d,t] = W^T x^T (contraction on partitions, N=512 moving).
    RMSNorm via ones-matmul partition reduction + Act Sqrt with the
    norm weight folded into scale/bias; RoPE via partition-swap DMA
    (SWDGE on the idle gpsimd queue) + DVE bf16 muls.
    Vn (natural [k,d]) via PE transpose.
  phase 2 (per q-slab J of 256, per head, kb groups of <=4 blocks):
    group: S^T = K Q^T (N=256 matmuls into a 2-bank PSUM tile), one
    Act exp over up to [128,1024] -> P bf16, DVE mask-mul on the
    diagonal tail, AV accumulation op += Vn^T P.
    Softmax denominator: near-free N=1 matmuls L[:,qc] += P_chunk^T
    @ ones (P is lhsT; L shares the op PSUM bank), then reciprocal ->
    PE transpose -> selector-matmul broadcast -> one DVE scale mul
    into OT. Out-projection chunks are interleaved between attention
    groups to keep PE saturated; output stores go out on the SP queue.
"""

import numpy as np

B, T, D = 2, 2048, 2048
NH, NKV, HD = 16, 4, 128
GQ = NH // NKV            # 4 q heads per kv head
HQ = GQ * HD              # 512 q-dim per core
ROPE_BASE = 500000.0
EPS = 1e-5
SCALE = 1.0 / np.sqrt(HD)
NQU = 4                   # phase-1 T quarters
QT_W = T // NQU           # 512
NDC = D // 128            # 16 contraction chunks
NJ = 8                    # phase-2 q slabs
JW = T // NJ              # 256
NKB = T // 128            # 16 k blocks
OCH = 512                 # out-projection D chunk

_cached = {}


def _build_program():
    import concourse.bacc as bacc
    import concourse.mybir as mybir
    from concourse import tile
    from concourse.masks import make_identity

    f32 = mybir.dt.float32
    f32r = mybir.dt.float32r
    bf16 = mybir.dt.bfloat16
    AF = mybir.ActivationFunctionType
    from concourse.bass import ts

    nc = bacc.Bacc()

    xt = nc.dram_tensor("xt", [D, T], bf16, kind="ExternalInput")
    wq = nc.dram_tensor("wq", [D, HQ], bf16, kind="ExternalInput")
    wk = nc.dram_tensor("wk", [D, HD], bf16, kind="ExternalInput")
    wv = nc.dram_tensor("wv", [D, HD], bf16, kind="ExternalInput")
    wo = nc.dram_tensor("wo", [HQ, D], bf16, kind="ExternalInput")
    cosd = nc.dram_tensor("cos", [HD, T], bf16, kind="ExternalInput")
    sind = nc.dram_tensor("sin", [HD, T], bf16, kind="ExternalInput")
    wqcd = nc.dram_tensor("wqc", [HD, 1], f32, kind="ExternalInput")
    wkcd = nc.dram_tensor("wkc", [HD, 1], f32, kind="ExternalInput")
    wqed = nc.dram_tensor("wqe", [HD, 1], f32, kind="ExternalInput")
    wked = nc.dram_tensor("wke", [HD, 1], f32, kind="ExternalInput")
    mskd = nc.dram_tensor("msk2", [HD, 2 * JW], bf16, kind="ExternalInput")
    onesnd = nc.dram_tensor("onesn", [128, 128], f32r, kind="ExternalInput")
    onescd = nc.dram_tensor("onesc", [128, 1], bf16, kind="ExternalInput")
    eseld = nc.dram_tensor("esel", [2, JW], bf16, kind="ExternalInput")
    outd = nc.dram_tensor("out", [T, D], f32, kind="ExternalOutput")

    xtr = xt.rearrange("(c p) t -> p c t", p=128)
    wqr = wq.rearrange("(c p) n -> p c n", p=128)
    wkr = wk.rearrange("(c p) n -> p c n", p=128)
    wvr = wv.rearrange("(c p) n -> p c n", p=128)
    wor = wo.rearrange("(c p) n -> p c n", p=128)
    mskr = mskd.rearrange("p (a q) -> p a q", a=2)

    with nc.allow_low_precision(reason="bf16 kernel, tolerance 2e-2"), \
         tile.TileContext(nc) as tc:
        with tc.tile_pool(name="A", bufs=1) as A, \
             tc.tile_pool(name="W", bufs=1) as W, \
             tc.tile_pool(name="BX", bufs=2) as BX:
            # persistent tensors and weights
            QT = A.tile([128, GQ, T], bf16, tag="QT")
            KT = A.tile([128, 1, T], bf16, tag="KT")
            Vn = A.tile([128, NKB, HD], bf16, tag="Vn")
            OT = A.tile([128, GQ, T], bf16, tag="OT")
            msk_sb = A.tile([128, 2, JW], bf16, tag="msk")
            onesn_sb = A.tile([128, 128], f32r, tag="onesn")
            onesc_sb = A.tile([128, 1], bf16, tag="onesc")
            esel_sb = A.tile([2, JW], bf16, tag="esel")
            ident_bf = A.tile([128, 128], bf16, tag="identb")
            wq_sb = W.tile([128, NDC, HQ], bf16, tag="wq")
            wk_sb = W.tile([128, NDC, HD], bf16, tag="wk")
            wv_sb = W.tile([128, NDC, HD], bf16, tag="wv")
            wo_sb = W.tile([128, GQ, D], bf16, tag="wo")
            cos_sb = A.tile([128, T], bf16, tag="cos")
            sin_sb = A.tile([128, T], bf16, tag="sin")
            wqc = A.tile([128, 1], f32, tag="wqc")
            wkc = A.tile([128, 1], f32, tag="wkc")
            wqe = A.tile([128, 1], f32, tag="wqe")
            wke = A.tile([128, 1], f32, tag="wke")

            # issue-order matters: K/V weights + first x quarter first
            xt_q = [None] * NQU
            xt_q0 = BX.tile([128, NDC, QT_W], bf16, tag="xt")
            xt_q[0] = xt_q0
            for c0 in range(0, NDC, 4):
                nc.sync.dma_start(out=wk_sb[:, c0:c0 + 4, :],
                                  in_=wkr[:, c0:c0 + 4, :])
                nc.sync.dma_start(out=xt_q0[:, c0:c0 + 4, :],
                                  in_=xtr[:, c0:c0 + 4, 0:QT_W])
            nc.sync.dma_start(out=wq_sb[:, 0:8, :], in_=wqr[:, 0:8, :])
            nc.sync.dma_start(out=wq_sb[:, 8:16, :], in_=wqr[:, 8:16, :])
            nc.sync.dma_start(out=cos_sb, in_=cosd[:, :])
            nc.sync.dma_start(out=sin_sb, in_=sind[:, :])
            nc.sync.dma_start(out=wv_sb, in_=wvr)
            nc.sync.dma_start(out=wqc, in_=wqcd[:, :])
            nc.sync.dma_start(out=wkc, in_=wkcd[:, :])
            nc.sync.dma_start(out=wqe, in_=wqed[:, :])
            nc.sync.dma_start(out=wke, in_=wked[:, :])
            nc.sync.dma_start(out=onesn_sb, in_=onesnd[:, :])
            xt_q1 = BX.tile([128, NDC, QT_W], bf16, tag="xt")
            xt_q[1] = xt_q1
            nc.sync.dma_start(out=xt_q1, in_=xtr[:, :, QT_W:2 * QT_W])
            nc.sync.dma_start(out=msk_sb, in_=mskr)
            nc.sync.dma_start(out=onesc_sb, in_=onescd[:, :])
            nc.sync.dma_start(out=esel_sb, in_=eseld[:, :])
            nc.sync.dma_start(out=wo_sb, in_=wor)
            make_identity(nc, ident_bf)

            # ---------------- phase 1: projections + norm + rope ----------
            with (
                tc.tile_pool(name="TMP", bufs=2) as TMP,
                tc.tile_pool(name="PKV", bufs=2, space="PSUM") as PKV,
                tc.tile_pool(name="PQ", bufs=3, space="PSUM") as PQ,
                tc.tile_pool(name="PL", bufs=1, space="PSUM") as PL,
                tc.tile_pool(name="PT", bufs=2, space="PSUM") as PT,
            ):
                def normrope(cpsum, wcol, wbias, sl, out_sl):
                    """RMSNorm + norm-weight + RoPE on a [128, 1, 512] PSUM
                    projection; writes bf16 out_sl [128, 1, 512]."""
                    sq = TMP.tile([128, 1, QT_W], f32r, tag="sq")
                    nc.scalar.activation(sq, cpsum, AF.Square)
                    l2 = PL.tile([128, 1, QT_W], f32, tag="l2")
                    nc.tensor.matmul(l2[:, 0, :], onesn_sb, sq[:, 0, :],
                                     start=True, stop=True)
                    sv = TMP.tile([128, 1, QT_W], f32, tag="sv")
                    nc.scalar.activation(sv, l2, AF.Sqrt, scale=wcol, bias=wbias)
                    rc = TMP.tile([128, 1, QT_W], f32, tag="rc")
                    nc.vector.reciprocal(rc, sv)
                    qn = TMP.tile([128, 1, QT_W], bf16, tag="qn")
                    nc.vector.tensor_mul(qn, cpsum, rc)
                    qr = TMP.tile([128, 1, QT_W], bf16, tag="qr")
                    nc.gpsimd.dma_start(out=qr[:64], in_=qn[64:])
                    nc.gpsimd.dma_start(out=qr[64:], in_=qn[:64])
                    t1 = TMP.tile([128, 1, QT_W], bf16, tag="t1")
                    t2 = TMP.tile([128, 1, QT_W], bf16, tag="t2")
                    nc.vector.tensor_mul(t1[:, 0, :], qn[:, 0, :], cos_sb[:, sl])
                    nc.vector.tensor_mul(t2[:, 0, :], qr[:, 0, :], sin_sb[:, sl])
                    nc.vector.tensor_sub(out_sl[:64], t1[:64], t2[:64])
                    nc.vector.tensor_add(out_sl[64:], t1[64:], t2[64:])

                for e in range(NQU):
                    sl = ts(e, QT_W)
                    if 1 <= e < NQU - 1:
                        xt_qn = BX.tile([128, NDC, QT_W], bf16, tag="xt")
                        xt_q[e + 1] = xt_qn
                        nc.sync.dma_start(out=xt_qn,
                                          in_=xtr[:, :, ts(e + 1, QT_W)])
                    xt_t = xt_q[e]
                    # K quarter
                    kp = PKV.tile([128, 1, QT_W], f32, tag="kv")
                    for c in range(NDC):
                        nc.tensor.matmul(kp[:, 0, :], wk_sb[:, c, :], xt_t[:, c, :],
                                         start=(c == 0), stop=(c == NDC - 1))
                    normrope(kp, wkc, wke, sl, KT[:, :, sl])
                    # Q heads
                    for h in range(GQ):
                        qp = PQ.tile([128, 1, QT_W], f32, tag="qp")
                        for c in range(NDC):
                            nc.tensor.matmul(qp[:, 0, :], wq_sb[:, c, ts(h, 128)],
                                             xt_t[:, c, :],
                                             start=(c == 0), stop=(c == NDC - 1))
                        normrope(qp, wqc, wqe, sl, QT[:, h:h + 1, sl])
                    # V last: its short tail covers the Q-normrope drain
                    vp = PKV.tile([128, 1, QT_W], f32, tag="kv")
                    for c in range(NDC):
                        nc.tensor.matmul(vp[:, 0, :], wv_sb[:, c, :], xt_t[:, c, :],
                                         start=(c == 0), stop=(c == NDC - 1))
                    vt = TMP.tile([128, QT_W], bf16, tag="vt")
                    nc.scalar.activation(vt, vp[:, 0, :], AF.Copy)
                    for i in range(QT_W // 128):
                        tp = PT.tile([128, 128], bf16, tag="tp")
                        nc.tensor.transpose(tp, vt[:, ts(i, 128)], ident_bf)
                        nc.scalar.activation(
                            Vn[:, e * (QT_W // 128) + i, :], tp, AF.Copy)

            # ---------------- phase 2: attention + out-projection ---------
            with (
                tc.tile_pool(name="CP", bufs=6) as CP,
                tc.tile_pool(name="CT", bufs=6) as CT,
                tc.tile_pool(name="CO", bufs=4) as CO,
                tc.tile_pool(name="SS", bufs=2, space="PSUM") as SS,
                tc.tile_pool(name="OPL", bufs=2, space="PSUM") as OPL,
                tc.tile_pool(name="PB", bufs=2, space="PSUM") as PB,
            ):
                pending = []  # deferred out-proj chunk emitters

                def emit_outproj_chunk(qt, cch):
                    oup = PB.tile([128, OCH], f32, tag="pb")
                    for hc in range(GQ):
                        nc.tensor.matmul(oup, OT[:, hc, ts(qt, 128)],
                                         wo_sb[:, hc, ts(cch, OCH)],
                                         start=(hc == 0), stop=(hc == GQ - 1))
                    ost = CO.tile([128, OCH], f32, tag="ost")
                    nc.scalar.copy(ost, oup)
                    nc.sync.dma_start(
                        out=outd[qt * 128:(qt + 1) * 128, ts(cch, OCH)], in_=ost)

                def emit_lchain(opl, lpacc, h, Jsl):
                    invL = CT.tile([128, 2], bf16, tag="invL")
                    nc.vector.reciprocal(invL, lpacc)
                    invB = PB.tile([128, OCH], f32, tag="pb")
                    # bf16 [2,128] scratch aliased into the unused top half
                    invLT_b = invB[0:2, 256:320].bitcast(bf16)
                    nc.tensor.transpose(invLT_b, invL, ident_bf)
                    invLT_sb = CT.tile([2, 128], bf16, tag="invLTs")
                    nc.vector.tensor_copy(invLT_sb, invLT_b)
                    for qc in range(2):
                        nc.tensor.matmul(invB[:, ts(qc, 128)],
                                         esel_sb[:, ts(qc, 128)],
                                         invLT_sb, start=True, stop=True)
                    invBs = CT.tile([128, JW], bf16, tag="invBs")
                    nc.vector.tensor_copy(invBs, invB[:, 0:JW])
                    nc.vector.tensor_mul(OT[:, h, Jsl], opl[:, 0:JW], invBs)

                deferred = []  # (emit_lchain closure, J, h) queue

                def queue_outproj(J):
                    for qt in range(2 * J, 2 * J + 2):
                        for cch in range(D // OCH):
                            pending.append(
                                lambda qt=qt, cch=cch: emit_outproj_chunk(qt, cch))

                def pop_fill():
                    # one deferred lchain per group slot, then out-proj pops
                    if deferred:
                        fn, dJ, dh = deferred.pop(0)
                        fn()
                        if dh == GQ - 1:
                            queue_outproj(dJ)
                    if pending:
                        pending.pop(0)()

                def emit_group(st, gi, gw):
                    """one kb-group of chain st = dict(J, Jsl, h, opl, lpacc,
                    kb, nkb, ngroups)"""
                    kb = st["kb"]
                    ss = SS.tile([128, 4, JW], f32, tag="ss")
                    for i in range(gw):
                        nc.tensor.matmul(ss[:, i, :],
                                         KT[:, 0, ts(kb + i, 128)],
                                         QT[:, st["h"], st["Jsl"]],
                                         start=True, stop=True)
                    P = CP.tile([128, 4, JW], bf16, tag="p")
                    nc.scalar.activation(P[:, 0:gw, :], ss[:, 0:gw, :],
                                         AF.Exp, scale=SCALE)
                    if gi == st["ngroups"] - 1:
                        # mask the diagonal pair (last 2 kb blocks)
                        nc.vector.tensor_mul(P[:, gw - 2:gw, :],
                                             P[:, gw - 2:gw, :], msk_sb)
                    # fill the exp-latency window with deferred work
                    pop_fill()
                    # group-local L mini-sums in the score tile's second bank
                    # (closed groups, qc-major), DVE-accumulated into SBUF
                    for qc in range(2):
                        for i in range(gw):
                            nc.tensor.matmul(ss[:, 3, qc:qc + 1],
                                             P[:, i, ts(qc, 128)], onesc_sb,
                                             start=(i == 0), stop=(i == gw - 1))
                    if gi == 0:
                        nc.vector.tensor_copy(st["lpacc"], ss[:, 3, 0:2])
                    else:
                        nc.vector.tensor_add(st["lpacc"], st["lpacc"],
                                             ss[:, 3, 0:2])
                    for i in range(gw):
                        nc.tensor.matmul(st["opl"][:, 0:JW], Vn[:, kb + i, :],
                                         P[:, i, :],
                                         start=(kb + i == 0),
                                         stop=(kb + i == st["nkb"] - 1))
                    st["kb"] = kb + gw

                def new_chain(J, h):
                    nkb = 2 * J + 2
                    opl = OPL.tile([128, OCH], f32, tag="opl")
                    lpacc = CT.tile([128, 2], f32, tag="lpacc")
                    groups = [4] * (nkb // 4) + ([2] if nkb % 4 else [])
                    return {"J": J, "Jsl": ts(J, JW), "h": h, "opl": opl,
                            "lpacc": lpacc, "kb": 0, "nkb": nkb,
                            "ngroups": len(groups), "groups": groups}

                # ascending J, heads interleaved in pairs
                for J in range(NJ):
                    for h0 in (0, 2):
                        ca, cb = new_chain(J, h0), new_chain(J, h0 + 1)
                        for gi in range(ca["ngroups"]):
                            emit_group(ca, gi, ca["groups"][gi])
                            emit_group(cb, gi, cb["groups"][gi])
                        for st in (ca, cb):
                            deferred.append((
                                (lambda st=st: emit_lchain(
                                    st["opl"], st["lpacc"], st["h"],
                                    st["Jsl"])), st["J"], st["h"]))
                while deferred or pending:
                    pop_fill()

    nc.finalize()
    return nc


def _host_consts():
    import ml_dtypes
    bf = ml_dtypes.bfloat16
    inv = 1.0 / (ROPE_BASE ** (np.arange(0, HD, 2, dtype=np.float64) / HD))
    freqs = np.outer(np.arange(T, dtype=np.float64), inv)
    emb = np.concatenate([freqs, freqs], axis=-1)          # [T, HD]
    cosT = np.ascontiguousarray(np.cos(emb).T.astype(np.float32)).astype(bf)
    sinT = np.ascontiguousarray(np.sin(emb).T.astype(np.float32)).astype(bf)
    # diagonal pair mask: msk2[p, i*JW + q] = (128*i + p <= q)
    msk2 = np.zeros((HD, 2 * JW), np.float32)
    for i in range(2):
        k = np.arange(128)[:, None] + 128 * i
        q = np.arange(JW)[None, :]
        msk2[:, i * JW:(i + 1) * JW] = (k <= q).astype(np.float32)
    msk2 = msk2.astype(bf)
    onesn = (np.ones((128, 128), np.float32) / HD)
    onesc = np.ones((128, 1), np.float32).astype(bf)
    esel = np.zeros((2, JW), np.float32)
    for qc in range(2):
        esel[qc, qc * 128:(qc + 1) * 128] = 1.0
    esel = esel.astype(bf)
    return cosT, sinT, msk2, onesn, onesc, esel


def kernel(x, Wq, Wk, Wv, Wo, q_norm_w, k_norm_w):
    import ml_dtypes
    from concourse.bass_utils import run_bass_kernel_spmd
    bf = ml_dtypes.bfloat16

    if "nc" not in _cached:
        _cached["nc"] = _build_program()
        _cached["consts"] = _host_consts()
    nc = _cached["nc"]
    cosT, sinT, msk2, onesn, onesc, esel = _cached["consts"]

    x = np.asarray(x, np.float32)
    Wq = np.asarray(Wq, np.float32)
    Wk = np.asarray(Wk, np.float32)
    Wv = np.asarray(Wv, np.float32)
    Wo = np.asarray(Wo, np.float32)
    qwf = np.asarray(q_norm_w, np.float64).reshape(HD, 1)
    kwf = np.asarray(k_norm_w, np.float64).reshape(HD, 1)
    qw = np.ascontiguousarray((1.0 / qwf ** 2).astype(np.float32))
    kw = np.ascontiguousarray((1.0 / kwf ** 2).astype(np.float32))
    qwe = np.ascontiguousarray((EPS / qwf ** 2).astype(np.float32))
    kwe = np.ascontiguousarray((EPS / kwf ** 2).astype(np.float32))

    xTb = [np.ascontiguousarray(x[b].T).astype(bf) for b in range(B)]
    in_maps = []
    for core in range(8):
        b, kv = divmod(core, NKV)
        in_maps.append({
            "xt": xTb[b],
            "wq": np.ascontiguousarray(Wq[:, kv * HQ:(kv + 1) * HQ]).astype(bf),
            "wk": np.ascontiguousarray(Wk[:, kv * HD:(kv + 1) * HD]).astype(bf),
            "wv": np.ascontiguousarray(Wv[:, kv * HD:(kv + 1) * HD]).astype(bf),
            "wo": np.ascontiguousarray(Wo[kv * HQ:(kv + 1) * HQ, :]).astype(bf),
            "cos": cosT, "sin": sinT,
            "wqc": qw, "wkc": kw, "wqe": qwe, "wke": kwe,
            "msk2": msk2, "onesn": onesn, "onesc": onesc, "esel": esel,
        })
    res = run_bass_kernel_spmd(nc, in_maps, list(range(8)))
    out = np.zeros((B, T, D), np.float64)
    for core in range(8):
        b = core // NKV
        out[b] += res.results[core]["out"].astype(np.float64)
    return out.astype(np.float32)


# revision 35
# speedup vs baseline: 1.4018x; 1.0026x over previous
"""GQA kernel for Trainium2, 8 NeuronCores.

Sharding: core c = b*4 + kv  (b in {0,1} data-parallel over batch,
kv in {0..3} tensor-parallel over the 4 KV head groups; each core owns
4 Q heads + 1 KV head). Each core computes a partial output
x[b] @ Wq[:,kv] -> attention -> @ Wo[kv rows]; host sums the 4 partials
per batch (the row-sharded-Wo all-reduce).

Device layout (per core), bf16 SBUF operands, f32 PSUM accumulation:
  phase 1 (per 512-col quarter of T, per head):
    KT/QT[d,t] = W^T x^T (contraction on partitions, N=512 moving).
    RMSNorm via ones-matmul partition reduction + Act Sqrt with the
    norm weight folded into scale/bias; RoPE via partition-swap DMA
    (SWDGE on the idle gpsimd queue) + DVE bf16 muls.
    Vn (natural [k,d]) via PE transpose.
  phase 2 (per q-slab J of 256, per head, kb groups of <=4 blocks):
    group: S^T = K Q^T (N=256 matmuls into a 2-bank PSUM tile), one
    Act exp over up to [128,1024] -> P bf16, DVE mask-mul on the
    diagonal tail, AV accumulation op += Vn^T P.
    Softmax denominator: near-free N=1 matmuls L[:,qc] += P_chunk^T
    @ ones (P is lhsT; L shares the op PSUM bank), then reciprocal ->
    PE transpose -> selector-matmul broadcast -> one DVE scale mul
    into OT. Out-projection chunks are interleaved between attention
    groups to keep PE saturated; output stores go out on the SP queue.
"""

import numpy as np

B, T, D = 2, 2048, 2048
NH, NKV, HD = 16, 4, 128
GQ = NH // NKV            # 4 q heads per kv head
HQ = GQ * HD              # 512 q-dim per core
ROPE_BASE = 500000.0
EPS = 1e-5
SCALE = 1.0 / np.sqrt(HD)
NQU = 4                   # phase-1 T quarters
QT_W = T // NQU           # 512
NDC = D // 128            # 16 contraction chunks
NJ = 8                    # phase-2 q slabs
JW = T // NJ              # 256
NKB = T // 128            # 16 k blocks
OCH = 512                 # out-projection D chunk

_cached = {}


def _build_program():
    import concourse.bacc as bacc
    import concourse.mybir as mybir
    from concourse import tile
    from concourse.masks import make_identity

    f32 = mybir.dt.float32
    f32r = mybir.dt.float32r
    bf16 = mybir.dt.bfloat16
    AF = mybir.ActivationFunctionType
    from concourse.bass import ts

    nc = bacc.Bacc()

    xt = nc.dram_tensor("xt", [D, T], bf16, kind="ExternalInput")
    wq = nc.dram_tensor("wq", [D, HQ], bf16, kind="ExternalInput")
    wk = nc.dram_tensor("wk", [D, HD], bf16, kind="ExternalInput")
    wv = nc.dram_tensor("wv", [D, HD], bf16, kind="ExternalInput")
    wo = nc.dram_tensor("wo", [HQ, D], bf16, kind="ExternalInput")
    cosd = nc.dram_tensor("cos", [HD, T], bf16, kind="ExternalInput")
    sind = nc.dram_tensor("sin", [HD, T], bf16, kind="ExternalInput")
    wqcd = nc.dram_tensor("wqc", [HD, 1], f32, kind="ExternalInput")
    wkcd = nc.dram_tensor("wkc", [HD, 1], f32, kind="ExternalInput")
    wqed = nc.dram_tensor("wqe", [HD, 1], f32, kind="ExternalInput")
    wked = nc.dram_tensor("wke", [HD, 1], f32, kind="ExternalInput")
    mskd = nc.dram_tensor("msk2", [HD, 2 * JW], bf16, kind="ExternalInput")
    onesnd = nc.dram_tensor("onesn", [128, 128], f32r, kind="ExternalInput")
    onescd = nc.dram_tensor("onesc", [128, 1], bf16, kind="ExternalInput")
    eseld = nc.dram_tensor("esel", [2, JW], bf16, kind="ExternalInput")
    outd = nc.dram_tensor("out", [T, D], f32, kind="ExternalOutput")

    xtr = xt.rearrange("(c p) t -> p c t", p=128)
    wqr = wq.rearrange("(c p) n -> p c n", p=128)
    wkr = wk.rearrange("(c p) n -> p c n", p=128)
    wvr = wv.rearrange("(c p) n -> p c n", p=128)
    wor = wo.rearrange("(c p) n -> p c n", p=128)
    mskr = mskd.rearrange("p (a q) -> p a q", a=2)

    with nc.allow_low_precision(reason="bf16 kernel, tolerance 2e-2"), \
         tile.TileContext(nc) as tc:
        with tc.tile_pool(name="A", bufs=1) as A, \
             tc.tile_pool(name="W", bufs=1) as W, \
             tc.tile_pool(name="BX", bufs=2) as BX:
            # persistent tensors and weights
            QT = A.tile([128, GQ, T], bf16, tag="QT")
            KT = A.tile([128, 1, T], bf16, tag="KT")
            Vn = A.tile([128, NKB, HD], bf16, tag="Vn")
            OT = A.tile([128, GQ, T], bf16, tag="OT")
            msk_sb = A.tile([128, 2, JW], bf16, tag="msk")
            onesn_sb = A.tile([128, 128], f32r, tag="onesn")
            onesc_sb = A.tile([128, 1], bf16, tag="onesc")
            esel_sb = A.tile([2, JW], bf16, tag="esel")
            ident_bf = A.tile([128, 128], bf16, tag="identb")
            wq_sb = W.tile([128, NDC, HQ], bf16, tag="wq")
            wk_sb = W.tile([128, NDC, HD], bf16, tag="wk")
            wv_sb = W.tile([128, NDC, HD], bf16, tag="wv")
            wo_sb = W.tile([128, GQ, D], bf16, tag="wo")
            cos_sb = A.tile([128, T], bf16, tag="cos")
            sin_sb = A.tile([128, T], bf16, tag="sin")
            wqc = A.tile([128, 1], f32, tag="wqc")
            wkc = A.tile([128, 1], f32, tag="wkc")
            wqe = A.tile([128, 1], f32, tag="wqe")
            wke = A.tile([128, 1], f32, tag="wke")

            # issue-order matters: K/V weights + first x quarter first
            xt_q = [None] * NQU
            xt_q0 = BX.tile([128, NDC, QT_W], bf16, tag="xt")
            xt_q[0] = xt_q0
            for c0 in range(0, NDC, 4):
                nc.sync.dma_start(out=wk_sb[:, c0:c0 + 4, :],
                                  in_=wkr[:, c0:c0 + 4, :])
                nc.sync.dma_start(out=xt_q0[:, c0:c0 + 4, :],
                                  in_=xtr[:, c0:c0 + 4, 0:QT_W])
            nc.sync.dma_start(out=wq_sb[:, 0:8, :], in_=wqr[:, 0:8, :])
            nc.sync.dma_start(out=wq_sb[:, 8:16, :], in_=wqr[:, 8:16, :])
            nc.sync.dma_start(out=cos_sb, in_=cosd[:, :])
            nc.sync.dma_start(out=sin_sb, in_=sind[:, :])
            nc.sync.dma_start(out=wv_sb, in_=wvr)
            nc.sync.dma_start(out=wqc, in_=wqcd[:, :])
            nc.sync.dma_start(out=wkc, in_=wkcd[:, :])
            nc.sync.dma_start(out=wqe, in_=wqed[:, :])
            nc.sync.dma_start(out=wke, in_=wked[:, :])
            nc.sync.dma_start(out=onesn_sb, in_=onesnd[:, :])
            xt_q1 = BX.tile([128, NDC, QT_W], bf16, tag="xt")
            xt_q[1] = xt_q1
            nc.sync.dma_start(out=xt_q1, in_=xtr[:, :, QT_W:2 * QT_W])
            nc.sync.dma_start(out=msk_sb, in_=mskr)
            nc.sync.dma_start(out=onesc_sb, in_=onescd[:, :])
            nc.sync.dma_start(out=esel_sb, in_=eseld[:, :])
            nc.sync.dma_start(out=wo_sb, in_=wor)
            make_identity(nc, ident_bf)

            # ---------------- phase 1: projections + norm + rope ----------
            with (
                tc.tile_pool(name="TMP", bufs=2) as TMP,
                tc.tile_pool(name="PKV", bufs=2, space="PSUM") as PKV,
                tc.tile_pool(name="PQ", bufs=3, space="PSUM") as PQ,
                tc.tile_pool(name="PL", bufs=1, space="PSUM") as PL,
                tc.tile_pool(name="PT", bufs=2, space="PSUM") as PT,
            ):
                def normrope(cpsum, wcol, wbias, sl, out_sl):
                    """RMSNorm + norm-weight + RoPE on a [128, 1, 512] PSUM
                    projection; writes bf16 out_sl [128, 1, 512]."""
                    sq = TMP.tile([128, 1, QT_W], f32r, tag="sq")
                    nc.scalar.activation(sq, cpsum, AF.Square)
                    l2 = PL.tile([128, 1, QT_W], f32, tag="l2")
                    nc.tensor.matmul(l2[:, 0, :], onesn_sb, sq[:, 0, :],
                                     start=True, stop=True)
                    sv = TMP.tile([128, 1, QT_W], f32, tag="sv")
                    nc.scalar.activation(sv, l2, AF.Sqrt, scale=wcol, bias=wbias)
                    rc = TMP.tile([128, 1, QT_W], f32, tag="rc")
                    nc.vector.reciprocal(rc, sv)
                    qn = TMP.tile([128, 1, QT_W], bf16, tag="qn")
                    nc.vector.tensor_mul(qn, cpsum, rc)
                    qr = TMP.tile([128, 1, QT_W], bf16, tag="qr")
                    nc.gpsimd.dma_start(out=qr[:64], in_=qn[64:])
                    nc.gpsimd.dma_start(out=qr[64:], in_=qn[:64])
                    t1 = TMP.tile([128, 1, QT_W], bf16, tag="t1")
                    t2 = TMP.tile([128, 1, QT_W], bf16, tag="t2")
                    nc.vector.tensor_mul(t1[:, 0, :], qn[:, 0, :], cos_sb[:, sl])
                    nc.vector.tensor_mul(t2[:, 0, :], qr[:, 0, :], sin_sb[:, sl])
                    nc.vector.tensor_sub(out_sl[:64], t1[:64], t2[:64])
                    nc.vector.tensor_add(out_sl[64:], t1[64:], t2[64:])

                for e in range(NQU):
                    sl = ts(e, QT_W)
                    if 1 <= e < NQU - 1:
                        xt_qn = BX.tile([128, NDC, QT_W], bf16, tag="xt")
                        xt_q[e + 1] = xt_qn
                        nc.sync.dma_start(out=xt_qn,
                                          in_=xtr[:, :, ts(e + 1, QT_W)])
                    xt_t = xt_q[e]
                    # K quarter
                    kp = PKV.tile([128, 1, QT_W], f32, tag="kv")
                    for c in range(NDC):
                        nc.tensor.matmul(kp[:, 0, :], wk_sb[:, c, :], xt_t[:, c, :],
                                         start=(c == 0), stop=(c == NDC - 1))
                    normrope(kp, wkc, wke, sl, KT[:, :, sl])
                    # Q heads
                    for h in range(GQ):
                        qp = PQ.tile([128, 1, QT_W], f32, tag="qp")
                        for c in range(NDC):
                            nc.tensor.matmul(qp[:, 0, :], wq_sb[:, c, ts(h, 128)],
                                             xt_t[:, c, :],
                                             start=(c == 0), stop=(c == NDC - 1))
                        normrope(qp, wqc, wqe, sl, QT[:, h:h + 1, sl])
                    # V last: its short tail covers the Q-normrope drain
                    vp = PKV.tile([128, 1, QT_W], f32, tag="kv")
                    for c in range(NDC):
                        nc.tensor.matmul(vp[:, 0, :], wv_sb[:, c, :], xt_t[:, c, :],
                                         start=(c == 0), stop=(c == NDC - 1))
                    vt = TMP.tile([128, QT_W], bf16, tag="vt")
                    nc.scalar.activation(vt, vp[:, 0, :], AF.Copy)
                    for i in range(QT_W // 128):
                        tp = PT.tile([128, 128], bf16, tag="tp")
                        nc.tensor.transpose(tp, vt[:, ts(i, 128)], ident_bf)
                        nc.scalar.activation(
                            Vn[:, e * (QT_W // 128) + i, :], tp, AF.Copy)

            # ---------------- phase 2: attention + out-projection ---------
            with (
                tc.tile_pool(name="CP", bufs=6) as CP,
                tc.tile_pool(name="CT", bufs=6) as CT,
                tc.tile_pool(name="CO", bufs=4) as CO,
                tc.tile_pool(name="SS", bufs=2, space="PSUM") as SS,
                tc.tile_pool(name="OPL", bufs=2, space="PSUM") as OPL,
                tc.tile_pool(name="PB", bufs=2, space="PSUM") as PB,
            ):
                pending = []  # deferred out-proj chunk emitters

                def emit_outproj_chunk(qt, cch):
                    oup = PB.tile([128, OCH], f32, tag="pb")
                    for hc in range(GQ):
                        nc.tensor.matmul(oup, OT[:, hc, ts(qt, 128)],
                                         wo_sb[:, hc, ts(cch, OCH)],
                                         start=(hc == 0), stop=(hc == GQ - 1))
                    ost = CO.tile([128, OCH], f32, tag="ost")
                    nc.scalar.copy(ost, oup)
                    nc.sync.dma_start(
                        out=outd[qt * 128:(qt + 1) * 128, ts(cch, OCH)], in_=ost)

                def emit_lchain(opl, lpacc, h, Jsl):
                    invL = CT.tile([128, 2], bf16, tag="invL")
                    nc.vector.reciprocal(invL, lpacc)
                    invB = PB.tile([128, OCH], f32, tag="pb")
                    # bf16 [2,128] scratch aliased into the unused top half
                    invLT_b = invB[0:2, 256:320].bitcast(bf16)
                    nc.tensor.transpose(invLT_b, invL, ident_bf)
                    invLT_sb = CT.tile([2, 128], bf16, tag="invLTs")
                    nc.vector.tensor_copy(invLT_sb, invLT_b)
                    for qc in range(2):
                        nc.tensor.matmul(invB[:, ts(qc, 128)],
                                         esel_sb[:, ts(qc, 128)],
                                         invLT_sb, start=True, stop=True)
                    invBs = CT.tile([128, JW], bf16, tag="invBs")
                    nc.vector.tensor_copy(invBs, invB[:, 0:JW])
                    nc.vector.tensor_mul(OT[:, h, Jsl], opl[:, 0:JW], invBs)

                deferred = []  # (emit_lchain closure, J, h) queue

                def queue_outproj(J):
                    for qt in range(2 * J, 2 * J + 2):
                        for cch in range(D // OCH):
                            pending.append(
                                lambda qt=qt, cch=cch: emit_outproj_chunk(qt, cch))

                def pop_fill():
                    # one deferred lchain per group slot, then out-proj pops
                    if deferred:
                        fn, dJ, dh = deferred.pop(0)
                        fn()
                        if dh == GQ - 1:
                            queue_outproj(dJ)
                    if pending:
                        pending.pop(0)()

                def emit_group(st, gi, gw):
                    """one kb-group of chain st = dict(J, Jsl, h, opl, lpacc,
                    kb, nkb, ngroups)"""
                    kb = st["kb"]
                    ss = SS.tile([128, 4, JW], f32, tag="ss")
                    for i in range(gw):
                        nc.tensor.matmul(ss[:, i, :],
                                         KT[:, 0, ts(kb + i, 128)],
                                         QT[:, st["h"], st["Jsl"]],
                                         start=True, stop=True)
                    P = CP.tile([128, 4, JW], bf16, tag="p")
                    nc.scalar.activation(P[:, 0:gw, :], ss[:, 0:gw, :],
                                         AF.Exp, scale=SCALE)
                    if gi == st["ngroups"] - 1:
                        # mask the diagonal pair (last 2 kb blocks)
                        nc.vector.tensor_mul(P[:, gw - 2:gw, :],
                                             P[:, gw - 2:gw, :], msk_sb)
                    # fill the exp-latency window with deferred work
                    pop_fill()
                    # group-local L mini-sums in the score tile's second bank
                    # (closed groups, qc-major), DVE-accumulated into SBUF;
                    # the diagonal group's last block is fully masked for the
                    # lower q-half -- skip its dead contributions
                    diag = gi == st["ngroups"] - 1
                    for qc in range(2):
                        for i in range(gw):
                            if diag and i == gw - 1 and qc == 0:
                                continue
                            nc.tensor.matmul(ss[:, 3, qc:qc + 1],
                                             P[:, i, ts(qc, 128)], onesc_sb,
                                             start=(i == 0),
                                             stop=(i == gw - 1 or
                                                   (diag and qc == 0 and
                                                    i == gw - 2)))
                    if gi == 0:
                        nc.vector.tensor_copy(st["lpacc"], ss[:, 3, 0:2])
                    else:
                        nc.vector.tensor_add(st["lpacc"], st["lpacc"],
                                             ss[:, 3, 0:2])
                    for i in range(gw):
                        if diag and i == gw - 1:
                            nc.tensor.matmul(st["opl"][:, 128:JW],
                                             Vn[:, kb + i, :],
                                             P[:, i, 128:JW],
                                             start=False, stop=True,
                                             skip_group_check=True)
                        else:
                            nc.tensor.matmul(st["opl"][:, 0:JW],
                                             Vn[:, kb + i, :], P[:, i, :],
                                             start=(kb + i == 0),
                                             stop=False,
                                             skip_group_check=True)
                    st["kb"] = kb + gw

                def new_chain(J, h):
                    nkb = 2 * J + 2
                    opl = OPL.tile([128, OCH], f32, tag="opl")
                    lpacc = CT.tile([128, 2], f32, tag="lpacc")
                    groups = [4] * (nkb // 4) + ([2] if nkb % 4 else [])
                    return {"J": J, "Jsl": ts(J, JW), "h": h, "opl": opl,
                            "lpacc": lpacc, "kb": 0, "nkb": nkb,
                            "ngroups": len(groups), "groups": groups}

                # ascending J, heads interleaved in pairs
                for J in range(NJ):
                    for h0 in (0, 2):
                        ca, cb = new_chain(J, h0), new_chain(J, h0 + 1)
                        for gi in range(ca["ngroups"]):
                            emit_group(ca, gi, ca["groups"][gi])
                            emit_group(cb, gi, cb["groups"][gi])
                        for st in (ca, cb):
                            deferred.append((
                                (lambda st=st: emit_lchain(
                                    st["opl"], st["lpacc"], st["h"],
                                    st["Jsl"])), st["J"], st["h"]))
                while deferred or pending:
                    pop_fill()

    nc.finalize()
    return nc


def _host_consts():
    import ml_dtypes
    bf = ml_dtypes.bfloat16
    inv = 1.0 / (ROPE_BASE ** (np.arange(0, HD, 2, dtype=np.float64) / HD))
    freqs = np.outer(np.arange(T, dtype=np.float64), inv)
    emb = np.concatenate([freqs, freqs], axis=-1)          # [T, HD]
    cosT = np.ascontiguousarray(np.cos(emb).T.astype(np.float32)).astype(bf)
    sinT = np.ascontiguousarray(np.sin(emb).T.astype(np.float32)).astype(bf)
    # diagonal pair mask: msk2[p, i*JW + q] = (128*i + p <= q)
    msk2 = np.zeros((HD, 2 * JW), np.float32)
    for i in range(2):
        k = np.arange(128)[:, None] + 128 * i
        q = np.arange(JW)[None, :]
        msk2[:, i * JW:(i + 1) * JW] = (k <= q).astype(np.float32)
    msk2 = msk2.astype(bf)
    onesn = (np.ones((128, 128), np.float32) / HD)
    onesc = np.ones((128, 1), np.float32).astype(bf)
    esel = np.zeros((2, JW), np.float32)
    for qc in range(2):
        esel[qc, qc * 128:(qc + 1) * 128] = 1.0
    esel = esel.astype(bf)
    return cosT, sinT, msk2, onesn, onesc, esel


def kernel(x, Wq, Wk, Wv, Wo, q_norm_w, k_norm_w):
    import ml_dtypes
    from concourse.bass_utils import run_bass_kernel_spmd
    bf = ml_dtypes.bfloat16

    if "nc" not in _cached:
        _cached["nc"] = _build_program()
        _cached["consts"] = _host_consts()
    nc = _cached["nc"]
    cosT, sinT, msk2, onesn, onesc, esel = _cached["consts"]

    x = np.asarray(x, np.float32)
    Wq = np.asarray(Wq, np.float32)
    Wk = np.asarray(Wk, np.float32)
    Wv = np.asarray(Wv, np.float32)
    Wo = np.asarray(Wo, np.float32)
    qwf = np.asarray(q_norm_w, np.float64).reshape(HD, 1)
    kwf = np.asarray(k_norm_w, np.float64).reshape(HD, 1)
    qw = np.ascontiguousarray((1.0 / qwf ** 2).astype(np.float32))
    kw = np.ascontiguousarray((1.0 / kwf ** 2).astype(np.float32))
    qwe = np.ascontiguousarray((EPS / qwf ** 2).astype(np.float32))
    kwe = np.ascontiguousarray((EPS / kwf ** 2).astype(np.float32))

    xTb = [np.ascontiguousarray(x[b].T).astype(bf) for b in range(B)]
    in_maps = []
    for core in range(8):
        b, kv = divmod(core, NKV)
        in_maps.append({
            "xt": xTb[b],
            "wq": np.ascontiguousarray(Wq[:, kv * HQ:(kv + 1) * HQ]).astype(bf),
            "wk": np.ascontiguousarray(Wk[:, kv * HD:(kv + 1) * HD]).astype(bf),
            "wv": np.ascontiguousarray(Wv[:, kv * HD:(kv + 1) * HD]).astype(bf),
            "wo": np.ascontiguousarray(Wo[kv * HQ:(kv + 1) * HQ, :]).astype(bf),
            "cos": cosT, "sin": sinT,
            "wqc": qw, "wkc": kw, "wqe": qwe, "wke": kwe,
            "msk2": msk2, "onesn": onesn, "onesc": onesc, "esel": esel,
        })
    res = run_bass_kernel_spmd(nc, in_maps, list(range(8)))
    out = np.zeros((B, T, D), np.float64)
    for core in range(8):
        b = core // NKV
        out[b] += res.results[core]["out"].astype(np.float64)
    return out.astype(np.float32)


# revision 36
# speedup vs baseline: 1.4038x; 1.0015x over previous
"""GQA kernel for Trainium2, 8 NeuronCores.

Sharding: core c = b*4 + kv  (b in {0,1} data-parallel over batch,
kv in {0..3} tensor-parallel over the 4 KV head groups; each core owns
4 Q heads + 1 KV head). Each core computes a partial output
x[b] @ Wq[:,kv] -> attention -> @ Wo[kv rows]; host sums the 4 partials
per batch (the row-sharded-Wo all-reduce).

Device layout (per core), bf16 SBUF operands, f32 PSUM accumulation:
  phase 1 (per 512-col quarter of T, per head):
    KT/QT[d,t] = W^T x^T (contraction on partitions, N=512 moving).
    RMSNorm via ones-matmul partition reduction + Act Sqrt with the
    norm weight folded into scale/bias; RoPE via partition-swap DMA
    (SWDGE on the idle gpsimd queue) + DVE bf16 muls.
    Vn (natural [k,d]) via PE transpose.
  phase 2 (per q-slab J of 256, per head, kb groups of <=4 blocks):
    group: S^T = K Q^T (N=256 matmuls into a 2-bank PSUM tile), one
    Act exp over up to [128,1024] -> P bf16, DVE mask-mul on the
    diagonal tail, AV accumulation op += Vn^T P.
    Softmax denominator: near-free N=1 matmuls L[:,qc] += P_chunk^T
    @ ones (P is lhsT; L shares the op PSUM bank), then reciprocal ->
    PE transpose -> selector-matmul broadcast -> one DVE scale mul
    into OT. Out-projection chunks are interleaved between attention
    groups to keep PE saturated; output stores go out on the SP queue.
"""

import numpy as np

B, T, D = 2, 2048, 2048
NH, NKV, HD = 16, 4, 128
GQ = NH // NKV            # 4 q heads per kv head
HQ = GQ * HD              # 512 q-dim per core
ROPE_BASE = 500000.0
EPS = 1e-5
SCALE = 1.0 / np.sqrt(HD)
NQU = 4                   # phase-1 T quarters
QT_W = T // NQU           # 512
NDC = D // 128            # 16 contraction chunks
NJ = 8                    # phase-2 q slabs
JW = T // NJ              # 256
NKB = T // 128            # 16 k blocks
OCH = 512                 # out-projection D chunk

_cached = {}


def _build_program():
    import concourse.bacc as bacc
    import concourse.mybir as mybir
    from concourse import tile
    from concourse.masks import make_identity

    f32 = mybir.dt.float32
    f32r = mybir.dt.float32r
    bf16 = mybir.dt.bfloat16
    AF = mybir.ActivationFunctionType
    from concourse.bass import ts

    nc = bacc.Bacc()

    xt = nc.dram_tensor("xt", [D, T], bf16, kind="ExternalInput")
    wq = nc.dram_tensor("wq", [D, HQ], bf16, kind="ExternalInput")
    wk = nc.dram_tensor("wk", [D, HD], bf16, kind="ExternalInput")
    wv = nc.dram_tensor("wv", [D, HD], bf16, kind="ExternalInput")
    wo = nc.dram_tensor("wo", [HQ, D], bf16, kind="ExternalInput")
    cosd = nc.dram_tensor("cos", [HD, T], bf16, kind="ExternalInput")
    sind = nc.dram_tensor("sin", [HD, T], bf16, kind="ExternalInput")
    wqcd = nc.dram_tensor("wqc", [HD, 1], f32, kind="ExternalInput")
    wkcd = nc.dram_tensor("wkc", [HD, 1], f32, kind="ExternalInput")
    wqed = nc.dram_tensor("wqe", [HD, 1], f32, kind="ExternalInput")
    wked = nc.dram_tensor("wke", [HD, 1], f32, kind="ExternalInput")
    mskd = nc.dram_tensor("msk2", [HD, 2 * JW], bf16, kind="ExternalInput")
    onesnd = nc.dram_tensor("onesn", [128, 128], f32r, kind="ExternalInput")
    onescd = nc.dram_tensor("onesc", [128, 1], bf16, kind="ExternalInput")
    eseld = nc.dram_tensor("esel", [2, JW], bf16, kind="ExternalInput")
    outd = nc.dram_tensor("out", [T, D], f32, kind="ExternalOutput")

    xtr = xt.rearrange("(c p) t -> p c t", p=128)
    wqr = wq.rearrange("(c p) n -> p c n", p=128)
    wkr = wk.rearrange("(c p) n -> p c n", p=128)
    wvr = wv.rearrange("(c p) n -> p c n", p=128)
    wor = wo.rearrange("(c p) n -> p c n", p=128)
    mskr = mskd.rearrange("p (a q) -> p a q", a=2)

    with nc.allow_low_precision(reason="bf16 kernel, tolerance 2e-2"), \
         tile.TileContext(nc) as tc:
        with tc.tile_pool(name="A", bufs=1) as A, \
             tc.tile_pool(name="W", bufs=1) as W, \
             tc.tile_pool(name="BX", bufs=2) as BX:
            # persistent tensors and weights
            QT = A.tile([128, GQ, T], bf16, tag="QT")
            KT = A.tile([128, 1, T], bf16, tag="KT")
            Vn = A.tile([128, NKB, HD], bf16, tag="Vn")
            OT = A.tile([128, GQ, T], bf16, tag="OT")
            msk_sb = A.tile([128, 2, JW], bf16, tag="msk")
            onesn_sb = A.tile([128, 128], f32r, tag="onesn")
            onesc_sb = A.tile([128, 1], bf16, tag="onesc")
            esel_sb = A.tile([2, JW], bf16, tag="esel")
            ident_bf = A.tile([128, 128], bf16, tag="identb")
            wq_sb = W.tile([128, NDC, HQ], bf16, tag="wq")
            wk_sb = W.tile([128, NDC, HD], bf16, tag="wk")
            wv_sb = W.tile([128, NDC, HD], bf16, tag="wv")
            wo_sb = W.tile([128, GQ, D], bf16, tag="wo")
            cos_sb = A.tile([128, T], bf16, tag="cos")
            sin_sb = A.tile([128, T], bf16, tag="sin")
            wqc = A.tile([128, 1], f32, tag="wqc")
            wkc = A.tile([128, 1], f32, tag="wkc")
            wqe = A.tile([128, 1], f32, tag="wqe")
            wke = A.tile([128, 1], f32, tag="wke")

            # issue-order matters: K/V weights + first x quarter first
            xt_q = [None] * NQU
            xt_q0 = BX.tile([128, NDC, QT_W], bf16, tag="xt")
            xt_q[0] = xt_q0
            for c0 in range(0, NDC, 4):
                nc.sync.dma_start(out=wk_sb[:, c0:c0 + 4, :],
                                  in_=wkr[:, c0:c0 + 4, :])
                nc.sync.dma_start(out=xt_q0[:, c0:c0 + 4, :],
                                  in_=xtr[:, c0:c0 + 4, 0:QT_W])
            nc.sync.dma_start(out=wq_sb[:, 0:8, :], in_=wqr[:, 0:8, :])
            nc.sync.dma_start(out=wq_sb[:, 8:16, :], in_=wqr[:, 8:16, :])
            nc.sync.dma_start(out=cos_sb, in_=cosd[:, :])
            nc.sync.dma_start(out=sin_sb, in_=sind[:, :])
            nc.sync.dma_start(out=wv_sb, in_=wvr)
            nc.sync.dma_start(out=wqc, in_=wqcd[:, :])
            nc.sync.dma_start(out=wkc, in_=wkcd[:, :])
            nc.sync.dma_start(out=wqe, in_=wqed[:, :])
            nc.sync.dma_start(out=wke, in_=wked[:, :])
            nc.sync.dma_start(out=onesn_sb, in_=onesnd[:, :])
            xt_q1 = BX.tile([128, NDC, QT_W], bf16, tag="xt")
            xt_q[1] = xt_q1
            nc.sync.dma_start(out=xt_q1, in_=xtr[:, :, QT_W:2 * QT_W])
            nc.sync.dma_start(out=msk_sb, in_=mskr)
            nc.sync.dma_start(out=onesc_sb, in_=onescd[:, :])
            nc.sync.dma_start(out=esel_sb, in_=eseld[:, :])
            nc.sync.dma_start(out=wo_sb, in_=wor)
            make_identity(nc, ident_bf)

            # ---------------- phase 1: projections + norm + rope ----------
            with (
                tc.tile_pool(name="TMP", bufs=2) as TMP,
                tc.tile_pool(name="PKV", bufs=2, space="PSUM") as PKV,
                tc.tile_pool(name="PQ", bufs=3, space="PSUM") as PQ,
                tc.tile_pool(name="PL", bufs=1, space="PSUM") as PL,
                tc.tile_pool(name="PT", bufs=2, space="PSUM") as PT,
            ):
                def normrope(cpsum, wcol, wbias, sl, out_sl):
                    """RMSNorm + norm-weight + RoPE on a [128, 1, 512] PSUM
                    projection; writes bf16 out_sl [128, 1, 512]."""
                    sq = TMP.tile([128, 1, QT_W], f32r, tag="sq")
                    nc.scalar.activation(sq, cpsum, AF.Square)
                    l2 = PL.tile([128, 1, QT_W], f32, tag="l2")
                    nc.tensor.matmul(l2[:, 0, :], onesn_sb, sq[:, 0, :],
                                     start=True, stop=True)
                    sv = TMP.tile([128, 1, QT_W], f32, tag="sv")
                    nc.scalar.activation(sv, l2, AF.Sqrt, scale=wcol, bias=wbias)
                    rc = TMP.tile([128, 1, QT_W], f32, tag="rc")
                    nc.vector.reciprocal(rc, sv)
                    qn = TMP.tile([128, 1, QT_W], bf16, tag="qn")
                    nc.vector.tensor_mul(qn, cpsum, rc)
                    qr = TMP.tile([128, 1, QT_W], bf16, tag="qr")
                    nc.gpsimd.dma_start(out=qr[:64], in_=qn[64:])
                    nc.gpsimd.dma_start(out=qr[64:], in_=qn[:64])
                    t1 = TMP.tile([128, 1, QT_W], bf16, tag="t1")
                    t2 = TMP.tile([128, 1, QT_W], bf16, tag="t2")
                    nc.vector.tensor_mul(t1[:, 0, :], qn[:, 0, :], cos_sb[:, sl])
                    nc.vector.tensor_mul(t2[:, 0, :], qr[:, 0, :], sin_sb[:, sl])
                    nc.vector.tensor_sub(out_sl[:64], t1[:64], t2[:64])
                    nc.vector.tensor_add(out_sl[64:], t1[64:], t2[64:])

                for e in range(NQU):
                    sl = ts(e, QT_W)
                    if 1 <= e < NQU - 1:
                        xt_qn = BX.tile([128, NDC, QT_W], bf16, tag="xt")
                        xt_q[e + 1] = xt_qn
                        nc.sync.dma_start(out=xt_qn,
                                          in_=xtr[:, :, ts(e + 1, QT_W)])
                    xt_t = xt_q[e]
                    # K quarter
                    kp = PKV.tile([128, 1, QT_W], f32, tag="kv")
                    for c in range(NDC):
                        nc.tensor.matmul(kp[:, 0, :], wk_sb[:, c, :], xt_t[:, c, :],
                                         start=(c == 0), stop=(c == NDC - 1))
                    normrope(kp, wkc, wke, sl, KT[:, :, sl])
                    # Q heads
                    for h in range(GQ):
                        qp = PQ.tile([128, 1, QT_W], f32, tag="qp")
                        for c in range(NDC):
                            nc.tensor.matmul(qp[:, 0, :], wq_sb[:, c, ts(h, 128)],
                                             xt_t[:, c, :],
                                             start=(c == 0), stop=(c == NDC - 1))
                        normrope(qp, wqc, wqe, sl, QT[:, h:h + 1, sl])
                    # V last: its short tail covers the Q-normrope drain
                    vp = PKV.tile([128, 1, QT_W], f32, tag="kv")
                    for c in range(NDC):
                        nc.tensor.matmul(vp[:, 0, :], wv_sb[:, c, :], xt_t[:, c, :],
                                         start=(c == 0), stop=(c == NDC - 1))
                    vt = TMP.tile([128, QT_W], bf16, tag="vt")
                    nc.scalar.activation(vt, vp[:, 0, :], AF.Copy)
                    for i in range(QT_W // 128):
                        tp = PT.tile([128, 128], bf16, tag="tp")
                        nc.tensor.transpose(tp, vt[:, ts(i, 128)], ident_bf)
                        nc.scalar.activation(
                            Vn[:, e * (QT_W // 128) + i, :], tp, AF.Copy)

            # ---------------- phase 2: attention + out-projection ---------
            with (
                tc.tile_pool(name="CP", bufs=6) as CP,
                tc.tile_pool(name="CT", bufs=6) as CT,
                tc.tile_pool(name="CO", bufs=4) as CO,
                tc.tile_pool(name="SS", bufs=2, space="PSUM") as SS,
                tc.tile_pool(name="OPL", bufs=2, space="PSUM") as OPL,
                tc.tile_pool(name="PB", bufs=2, space="PSUM") as PB,
            ):
                pending = []  # deferred out-proj chunk emitters

                def emit_outproj_chunk(qt, cch):
                    oup = PB.tile([128, OCH], f32, tag="pb")
                    for hc in range(GQ):
                        nc.tensor.matmul(oup, OT[:, hc, ts(qt, 128)],
                                         wo_sb[:, hc, ts(cch, OCH)],
                                         start=(hc == 0), stop=(hc == GQ - 1))
                    ost = CO.tile([128, OCH], f32, tag="ost")
                    nc.scalar.copy(ost, oup)
                    nc.sync.dma_start(
                        out=outd[qt * 128:(qt + 1) * 128, ts(cch, OCH)], in_=ost)

                def emit_lchain(opl, lpacc, h, Jsl):
                    invL = CT.tile([128, 2], bf16, tag="invL")
                    nc.vector.reciprocal(invL, lpacc)
                    invB = PB.tile([128, OCH], f32, tag="pb")
                    # bf16 [2,128] scratch aliased into the unused top half
                    invLT_b = invB[0:2, 256:320].bitcast(bf16)
                    nc.tensor.transpose(invLT_b, invL, ident_bf)
                    invLT_sb = CT.tile([2, 128], bf16, tag="invLTs")
                    nc.vector.tensor_copy(invLT_sb, invLT_b)
                    for qc in range(2):
                        nc.tensor.matmul(invB[:, ts(qc, 128)],
                                         esel_sb[:, ts(qc, 128)],
                                         invLT_sb, start=True, stop=True)
                    invBs = CT.tile([128, JW], bf16, tag="invBs")
                    nc.vector.tensor_copy(invBs, invB[:, 0:JW])
                    nc.vector.tensor_mul(OT[:, h, Jsl], opl[:, 0:JW], invBs)

                deferred = []  # (emit_lchain closure, J, h) queue

                def queue_outproj(J):
                    for qt in range(2 * J, 2 * J + 2):
                        for cch in range(D // OCH):
                            pending.append(
                                lambda qt=qt, cch=cch: emit_outproj_chunk(qt, cch))

                slot_ctr = {"n": 0}

                def pop_fill():
                    # one deferred lchain per group slot, then out-proj pops;
                    # in the last J the backlog is scarce -- spread pops
                    slot_ctr["n"] += 1
                    if deferred:
                        fn, dJ, dh = deferred.pop(0)
                        fn()
                        if dh == GQ - 1:
                            queue_outproj(dJ)
                    if pending:
                        if len(pending) <= 8 and slot_ctr.get("lastJ") \
                                and slot_ctr["n"] % 2 == 0:
                            return
                        pending.pop(0)()

                def emit_group(st, gi, gw):
                    """one kb-group of chain st = dict(J, Jsl, h, opl, lpacc,
                    kb, nkb, ngroups)"""
                    kb = st["kb"]
                    ss = SS.tile([128, 4, JW], f32, tag="ss")
                    for i in range(gw):
                        nc.tensor.matmul(ss[:, i, :],
                                         KT[:, 0, ts(kb + i, 128)],
                                         QT[:, st["h"], st["Jsl"]],
                                         start=True, stop=True)
                    P = CP.tile([128, 4, JW], bf16, tag="p")
                    nc.scalar.activation(P[:, 0:gw, :], ss[:, 0:gw, :],
                                         AF.Exp, scale=SCALE)
                    if gi == st["ngroups"] - 1:
                        # mask the diagonal pair (last 2 kb blocks)
                        nc.vector.tensor_mul(P[:, gw - 2:gw, :],
                                             P[:, gw - 2:gw, :], msk_sb)
                    # fill the exp-latency window with deferred work
                    pop_fill()
                    # group-local L mini-sums in the score tile's second bank
                    # (closed groups, qc-major), DVE-accumulated into SBUF;
                    # the diagonal group's last block is fully masked for the
                    # lower q-half -- skip its dead contributions
                    diag = gi == st["ngroups"] - 1
                    for qc in range(2):
                        for i in range(gw):
                            if diag and i == gw - 1 and qc == 0:
                                continue
                            nc.tensor.matmul(ss[:, 3, qc:qc + 1],
                                             P[:, i, ts(qc, 128)], onesc_sb,
                                             start=(i == 0),
                                             stop=(i == gw - 1 or
                                                   (diag and qc == 0 and
                                                    i == gw - 2)))
                    if gi == 0:
                        nc.vector.tensor_copy(st["lpacc"], ss[:, 3, 0:2])
                    else:
                        nc.vector.tensor_add(st["lpacc"], st["lpacc"],
                                             ss[:, 3, 0:2])
                    for i in range(gw):
                        if diag and i == gw - 1:
                            nc.tensor.matmul(st["opl"][:, 128:JW],
                                             Vn[:, kb + i, :],
                                             P[:, i, 128:JW],
                                             start=False, stop=True,
                                             skip_group_check=True)
                        else:
                            nc.tensor.matmul(st["opl"][:, 0:JW],
                                             Vn[:, kb + i, :], P[:, i, :],
                                             start=(kb + i == 0),
                                             stop=False,
                                             skip_group_check=True)
                    st["kb"] = kb + gw

                def new_chain(J, h):
                    nkb = 2 * J + 2
                    opl = OPL.tile([128, OCH], f32, tag="opl")
                    lpacc = CT.tile([128, 2], f32, tag="lpacc")
                    groups = [4] * (nkb // 4) + ([2] if nkb % 4 else [])
                    return {"J": J, "Jsl": ts(J, JW), "h": h, "opl": opl,
                            "lpacc": lpacc, "kb": 0, "nkb": nkb,
                            "ngroups": len(groups), "groups": groups}

                # ascending J, heads interleaved in pairs
                for J in range(NJ):
                    slot_ctr["lastJ"] = (J == NJ - 1)
                    for h0 in (0, 2):
                        ca, cb = new_chain(J, h0), new_chain(J, h0 + 1)
                        for gi in range(ca["ngroups"]):
                            emit_group(ca, gi, ca["groups"][gi])
                            emit_group(cb, gi, cb["groups"][gi])
                        for st in (ca, cb):
                            deferred.append((
                                (lambda st=st: emit_lchain(
                                    st["opl"], st["lpacc"], st["h"],
                                    st["Jsl"])), st["J"], st["h"]))
                while deferred or pending:
                    pop_fill()

    nc.finalize()
    return nc


def _host_consts():
    import ml_dtypes
    bf = ml_dtypes.bfloat16
    inv = 1.0 / (ROPE_BASE ** (np.arange(0, HD, 2, dtype=np.float64) / HD))
    freqs = np.outer(np.arange(T, dtype=np.float64), inv)
    emb = np.concatenate([freqs, freqs], axis=-1)          # [T, HD]
    cosT = np.ascontiguousarray(np.cos(emb).T.astype(np.float32)).astype(bf)
    sinT = np.ascontiguousarray(np.sin(emb).T.astype(np.float32)).astype(bf)
    # diagonal pair mask: msk2[p, i*JW + q] = (128*i + p <= q)
    msk2 = np.zeros((HD, 2 * JW), np.float32)
    for i in range(2):
        k = np.arange(128)[:, None] + 128 * i
        q = np.arange(JW)[None, :]
        msk2[:, i * JW:(i + 1) * JW] = (k <= q).astype(np.float32)
    msk2 = msk2.astype(bf)
    onesn = (np.ones((128, 128), np.float32) / HD)
    onesc = np.ones((128, 1), np.float32).astype(bf)
    esel = np.zeros((2, JW), np.float32)
    for qc in range(2):
        esel[qc, qc * 128:(qc + 1) * 128] = 1.0
    esel = esel.astype(bf)
    return cosT, sinT, msk2, onesn, onesc, esel


def kernel(x, Wq, Wk, Wv, Wo, q_norm_w, k_norm_w):
    import ml_dtypes
    from concourse.bass_utils import run_bass_kernel_spmd
    bf = ml_dtypes.bfloat16

    if "nc" not in _cached:
        _cached["nc"] = _build_program()
        _cached["consts"] = _host_consts()
    nc = _cached["nc"]
    cosT, sinT, msk2, onesn, onesc, esel = _cached["consts"]

    x = np.asarray(x, np.float32)
    Wq = np.asarray(Wq, np.float32)
    Wk = np.asarray(Wk, np.float32)
    Wv = np.asarray(Wv, np.float32)
    Wo = np.asarray(Wo, np.float32)
    qwf = np.asarray(q_norm_w, np.float64).reshape(HD, 1)
    kwf = np.asarray(k_norm_w, np.float64).reshape(HD, 1)
    qw = np.ascontiguousarray((1.0 / qwf ** 2).astype(np.float32))
    kw = np.ascontiguousarray((1.0 / kwf ** 2).astype(np.float32))
    qwe = np.ascontiguousarray((EPS / qwf ** 2).astype(np.float32))
    kwe = np.ascontiguousarray((EPS / kwf ** 2).astype(np.float32))

    xTb = [np.ascontiguousarray(x[b].T).astype(bf) for b in range(B)]
    in_maps = []
    for core in range(8):
        b, kv = divmod(core, NKV)
        in_maps.append({
            "xt": xTb[b],
            "wq": np.ascontiguousarray(Wq[:, kv * HQ:(kv + 1) * HQ]).astype(bf),
            "wk": np.ascontiguousarray(Wk[:, kv * HD:(kv + 1) * HD]).astype(bf),
            "wv": np.ascontiguousarray(Wv[:, kv * HD:(kv + 1) * HD]).astype(bf),
            "wo": np.ascontiguousarray(Wo[kv * HQ:(kv + 1) * HQ, :]).astype(bf),
            "cos": cosT, "sin": sinT,
            "wqc": qw, "wkc": kw, "wqe": qwe, "wke": kwe,
            "msk2": msk2, "onesn": onesn, "onesc": onesc, "esel": esel,
        })
    res = run_bass_kernel_spmd(nc, in_maps, list(range(8)))
    out = np.zeros((B, T, D), np.float64)
    for core in range(8):
        b = core // NKV
        out[b] += res.results[core]["out"].astype(np.float64)
    return out.astype(np.float32)


# revision 37
# speedup vs baseline: 1.4112x; 1.0052x over previous
"""GQA kernel for Trainium2, 8 NeuronCores.

Sharding: core c = b*4 + kv  (b in {0,1} data-parallel over batch,
kv in {0..3} tensor-parallel over the 4 KV head groups; each core owns
4 Q heads + 1 KV head). Each core computes a partial output
x[b] @ Wq[:,kv] -> attention -> @ Wo[kv rows]; host sums the 4 partials
per batch (the row-sharded-Wo all-reduce).

Device layout (per core), bf16 SBUF operands, f32 PSUM accumulation:
  phase 1 (per 512-col quarter of T, per head):
    KT/QT[d,t] = W^T x^T (contraction on partitions, N=512 moving).
    RMSNorm via ones-matmul partition reduction + Act Sqrt with the
    norm weight folded into scale/bias; RoPE via partition-swap DMA
    (SWDGE on the idle gpsimd queue) + DVE bf16 muls.
    Vn (natural [k,d]) via PE transpose.
  phase 2 (per q-slab J of 256, per head, kb groups of <=4 blocks):
    group: S^T = K Q^T (N=256 matmuls into a 2-bank PSUM tile), one
    Act exp over up to [128,1024] -> P bf16, DVE mask-mul on the
    diagonal tail, AV accumulation op += Vn^T P.
    Softmax denominator: near-free N=1 matmuls L[:,qc] += P_chunk^T
    @ ones (P is lhsT; L shares the op PSUM bank), then reciprocal ->
    PE transpose -> selector-matmul broadcast -> one DVE scale mul
    into OT. Out-projection chunks are interleaved between attention
    groups to keep PE saturated; output stores go out on the SP queue.
"""

import numpy as np

B, T, D = 2, 2048, 2048
NH, NKV, HD = 16, 4, 128
GQ = NH // NKV            # 4 q heads per kv head
HQ = GQ * HD              # 512 q-dim per core
ROPE_BASE = 500000.0
EPS = 1e-5
SCALE = 1.0 / np.sqrt(HD)
NQU = 4                   # phase-1 T quarters
QT_W = T // NQU           # 512
NDC = D // 128            # 16 contraction chunks
NJ = 8                    # phase-2 q slabs
JW = T // NJ              # 256
NKB = T // 128            # 16 k blocks
OCH = 512                 # out-projection D chunk

_cached = {}


def _build_program():
    import concourse.bacc as bacc
    import concourse.mybir as mybir
    from concourse import tile
    from concourse.masks import make_identity

    f32 = mybir.dt.float32
    f32r = mybir.dt.float32r
    bf16 = mybir.dt.bfloat16
    AF = mybir.ActivationFunctionType
    from concourse.bass import ts

    nc = bacc.Bacc()

    xt = nc.dram_tensor("xt", [D, T], bf16, kind="ExternalInput")
    wq = nc.dram_tensor("wq", [D, HQ], bf16, kind="ExternalInput")
    wk = nc.dram_tensor("wk", [D, HD], bf16, kind="ExternalInput")
    wv = nc.dram_tensor("wv", [D, HD], bf16, kind="ExternalInput")
    wo = nc.dram_tensor("wo", [HQ, D], bf16, kind="ExternalInput")
    cosd = nc.dram_tensor("cos", [HD, T], bf16, kind="ExternalInput")
    sind = nc.dram_tensor("sin", [HD, T], bf16, kind="ExternalInput")
    wqcd = nc.dram_tensor("wqc", [HD, 1], f32, kind="ExternalInput")
    wkcd = nc.dram_tensor("wkc", [HD, 1], f32, kind="ExternalInput")
    wqed = nc.dram_tensor("wqe", [HD, 1], f32, kind="ExternalInput")
    wked = nc.dram_tensor("wke", [HD, 1], f32, kind="ExternalInput")
    mskd = nc.dram_tensor("msk2", [HD, 2 * JW], bf16, kind="ExternalInput")
    onesnd = nc.dram_tensor("onesn", [128, 128], f32r, kind="ExternalInput")
    onescd = nc.dram_tensor("onesc", [128, 1], bf16, kind="ExternalInput")
    eseld = nc.dram_tensor("esel", [2, JW], bf16, kind="ExternalInput")
    outd = nc.dram_tensor("out", [T, D], f32, kind="ExternalOutput")

    xtr = xt.rearrange("(c p) t -> p c t", p=128)
    wqr = wq.rearrange("(c p) n -> p c n", p=128)
    wkr = wk.rearrange("(c p) n -> p c n", p=128)
    wvr = wv.rearrange("(c p) n -> p c n", p=128)
    wor = wo.rearrange("(c p) n -> p c n", p=128)
    mskr = mskd.rearrange("p (a q) -> p a q", a=2)

    with nc.allow_low_precision(reason="bf16 kernel, tolerance 2e-2"), \
         tile.TileContext(nc) as tc:
        with tc.tile_pool(name="A", bufs=1) as A, \
             tc.tile_pool(name="W", bufs=1) as W, \
             tc.tile_pool(name="BX", bufs=2) as BX:
            # persistent tensors and weights
            QT = A.tile([128, GQ, T], bf16, tag="QT")
            KT = A.tile([128, 1, T], bf16, tag="KT")
            Vn = A.tile([128, NKB, HD], bf16, tag="Vn")
            OT = A.tile([128, GQ, T], bf16, tag="OT")
            msk_sb = A.tile([128, 2, JW], bf16, tag="msk")
            onesn_sb = A.tile([128, 128], f32r, tag="onesn")
            onesc_sb = A.tile([128, 1], bf16, tag="onesc")
            esel_sb = A.tile([2, JW], bf16, tag="esel")
            ident_bf = A.tile([128, 128], bf16, tag="identb")
            wq_sb = W.tile([128, NDC, HQ], bf16, tag="wq")
            wk_sb = W.tile([128, NDC, HD], bf16, tag="wk")
            wv_sb = W.tile([128, NDC, HD], bf16, tag="wv")
            wo_sb = W.tile([128, GQ, D], bf16, tag="wo")
            cos_sb = A.tile([128, T], bf16, tag="cos")
            sin_sb = A.tile([128, T], bf16, tag="sin")
            wqc = A.tile([128, 1], f32, tag="wqc")
            wkc = A.tile([128, 1], f32, tag="wkc")
            wqe = A.tile([128, 1], f32, tag="wqe")
            wke = A.tile([128, 1], f32, tag="wke")

            # issue-order matters: K/V weights + first x quarter first
            xt_q = [None] * NQU
            xt_q0 = BX.tile([128, NDC, QT_W], bf16, tag="xt")
            xt_q[0] = xt_q0
            for c0 in range(0, NDC, 4):
                nc.sync.dma_start(out=wk_sb[:, c0:c0 + 4, :],
                                  in_=wkr[:, c0:c0 + 4, :])
                nc.sync.dma_start(out=xt_q0[:, c0:c0 + 4, :],
                                  in_=xtr[:, c0:c0 + 4, 0:QT_W])
            nc.sync.dma_start(out=wq_sb[:, 0:8, :], in_=wqr[:, 0:8, :])
            nc.sync.dma_start(out=wq_sb[:, 8:16, :], in_=wqr[:, 8:16, :])
            nc.sync.dma_start(out=cos_sb, in_=cosd[:, :])
            nc.sync.dma_start(out=sin_sb, in_=sind[:, :])
            nc.sync.dma_start(out=wv_sb, in_=wvr)
            nc.sync.dma_start(out=wqc, in_=wqcd[:, :])
            nc.sync.dma_start(out=wkc, in_=wkcd[:, :])
            nc.sync.dma_start(out=wqe, in_=wqed[:, :])
            nc.sync.dma_start(out=wke, in_=wked[:, :])
            nc.sync.dma_start(out=onesn_sb, in_=onesnd[:, :])
            xt_q1 = BX.tile([128, NDC, QT_W], bf16, tag="xt")
            xt_q[1] = xt_q1
            nc.sync.dma_start(out=xt_q1, in_=xtr[:, :, QT_W:2 * QT_W])
            nc.sync.dma_start(out=msk_sb, in_=mskr)
            nc.sync.dma_start(out=onesc_sb, in_=onescd[:, :])
            nc.sync.dma_start(out=esel_sb, in_=eseld[:, :])
            nc.sync.dma_start(out=wo_sb, in_=wor)
            make_identity(nc, ident_bf)

            # ---------------- phase 1: projections + norm + rope ----------
            with (
                tc.tile_pool(name="TMP", bufs=2) as TMP,
                tc.tile_pool(name="PKV", bufs=2, space="PSUM") as PKV,
                tc.tile_pool(name="PQ", bufs=3, space="PSUM") as PQ,
                tc.tile_pool(name="PL", bufs=1, space="PSUM") as PL,
                tc.tile_pool(name="PT", bufs=2, space="PSUM") as PT,
            ):
                def normrope(cpsum, wcol, wbias, sl, out_sl):
                    """RMSNorm + norm-weight + RoPE on a [128, 1, 512] PSUM
                    projection; writes bf16 out_sl [128, 1, 512]."""
                    sq = TMP.tile([128, 1, QT_W], f32r, tag="sq")
                    nc.scalar.activation(sq, cpsum, AF.Square)
                    l2 = PL.tile([128, 1, QT_W], f32, tag="l2")
                    nc.tensor.matmul(l2[:, 0, :], onesn_sb, sq[:, 0, :],
                                     start=True, stop=True)
                    sv = TMP.tile([128, 1, QT_W], f32, tag="sv")
                    nc.scalar.activation(sv, l2, AF.Sqrt, scale=wcol, bias=wbias)
                    rc = TMP.tile([128, 1, QT_W], f32, tag="rc")
                    nc.vector.reciprocal(rc, sv)
                    qn = TMP.tile([128, 1, QT_W], bf16, tag="qn")
                    nc.vector.tensor_mul(qn, cpsum, rc)
                    qr = TMP.tile([128, 1, QT_W], bf16, tag="qr")
                    nc.gpsimd.dma_start(out=qr[:64], in_=qn[64:])
                    nc.gpsimd.dma_start(out=qr[64:], in_=qn[:64])
                    t1 = TMP.tile([128, 1, QT_W], bf16, tag="t1")
                    t2 = TMP.tile([128, 1, QT_W], bf16, tag="t2")
                    nc.vector.tensor_mul(t1[:, 0, :], qn[:, 0, :], cos_sb[:, sl])
                    nc.vector.tensor_mul(t2[:, 0, :], qr[:, 0, :], sin_sb[:, sl])
                    nc.vector.tensor_sub(out_sl[:64], t1[:64], t2[:64])
                    nc.vector.tensor_add(out_sl[64:], t1[64:], t2[64:])

                for e in range(NQU):
                    sl = ts(e, QT_W)
                    if 1 <= e < NQU - 1:
                        xt_qn = BX.tile([128, NDC, QT_W], bf16, tag="xt")
                        xt_q[e + 1] = xt_qn
                        nc.sync.dma_start(out=xt_qn,
                                          in_=xtr[:, :, ts(e + 1, QT_W)])
                    xt_t = xt_q[e]
                    # K quarter
                    kp = PKV.tile([128, 1, QT_W], f32, tag="kv")
                    for c in range(NDC):
                        nc.tensor.matmul(kp[:, 0, :], wk_sb[:, c, :], xt_t[:, c, :],
                                         start=(c == 0), stop=(c == NDC - 1))
                    normrope(kp, wkc, wke, sl, KT[:, :, sl])
                    # Q heads
                    for h in range(GQ):
                        qp = PQ.tile([128, 1, QT_W], f32, tag="qp")
                        for c in range(NDC):
                            nc.tensor.matmul(qp[:, 0, :], wq_sb[:, c, ts(h, 128)],
                                             xt_t[:, c, :],
                                             start=(c == 0), stop=(c == NDC - 1))
                        normrope(qp, wqc, wqe, sl, QT[:, h:h + 1, sl])
                    # V last: its short tail covers the Q-normrope drain
                    vp = PKV.tile([128, 1, QT_W], f32, tag="kv")
                    for c in range(NDC):
                        nc.tensor.matmul(vp[:, 0, :], wv_sb[:, c, :], xt_t[:, c, :],
                                         start=(c == 0), stop=(c == NDC - 1))
                    vt = TMP.tile([128, QT_W], bf16, tag="vt")
                    nc.scalar.activation(vt, vp[:, 0, :], AF.Copy)
                    for i in range(QT_W // 128):
                        tp = PT.tile([128, 128], bf16, tag="tp")
                        nc.tensor.transpose(tp, vt[:, ts(i, 128)], ident_bf)
                        nc.scalar.activation(
                            Vn[:, e * (QT_W // 128) + i, :], tp, AF.Copy)
                    if e == NQU - 1:
                        # pull the Exp table load after the final copies so
                        # it overlaps the normrope drain, not the handover
                        wuex = TMP.tile([128, 1, QT_W], f32, tag="sq")
                        nc.scalar.activation(wuex[:, 0, 0:1], vt[:, 0:1],
                                             AF.Exp)

            # ---------------- phase 2: attention + out-projection ---------
            with (
                tc.tile_pool(name="CP", bufs=6) as CP,
                tc.tile_pool(name="CT", bufs=6) as CT,
                tc.tile_pool(name="CO", bufs=4) as CO,
                tc.tile_pool(name="SS", bufs=2, space="PSUM") as SS,
                tc.tile_pool(name="OPL", bufs=2, space="PSUM") as OPL,
                tc.tile_pool(name="PB", bufs=2, space="PSUM") as PB,
            ):
                pending = []  # deferred out-proj chunk emitters

                def emit_outproj_chunk(qt, cch):
                    oup = PB.tile([128, OCH], f32, tag="pb")
                    for hc in range(GQ):
                        nc.tensor.matmul(oup, OT[:, hc, ts(qt, 128)],
                                         wo_sb[:, hc, ts(cch, OCH)],
                                         start=(hc == 0), stop=(hc == GQ - 1))
                    ost = CO.tile([128, OCH], f32, tag="ost")
                    nc.scalar.copy(ost, oup)
                    nc.sync.dma_start(
                        out=outd[qt * 128:(qt + 1) * 128, ts(cch, OCH)], in_=ost)

                def emit_lchain(opl, lpacc, h, Jsl):
                    invL = CT.tile([128, 2], bf16, tag="invL")
                    nc.vector.reciprocal(invL, lpacc)
                    invB = PB.tile([128, OCH], f32, tag="pb")
                    # bf16 [2,128] scratch aliased into the unused top half
                    invLT_b = invB[0:2, 256:320].bitcast(bf16)
                    nc.tensor.transpose(invLT_b, invL, ident_bf)
                    invLT_sb = CT.tile([2, 128], bf16, tag="invLTs")
                    nc.vector.tensor_copy(invLT_sb, invLT_b)
                    for qc in range(2):
                        nc.tensor.matmul(invB[:, ts(qc, 128)],
                                         esel_sb[:, ts(qc, 128)],
                                         invLT_sb, start=True, stop=True)
                    invBs = CT.tile([128, JW], bf16, tag="invBs")
                    nc.vector.tensor_copy(invBs, invB[:, 0:JW])
                    nc.vector.tensor_mul(OT[:, h, Jsl], opl[:, 0:JW], invBs)

                deferred = []  # (emit_lchain closure, J, h) queue

                def queue_outproj(J):
                    for qt in range(2 * J, 2 * J + 2):
                        for cch in range(D // OCH):
                            pending.append(
                                lambda qt=qt, cch=cch: emit_outproj_chunk(qt, cch))

                slot_ctr = {"n": 0}

                def pop_fill():
                    # one deferred lchain per group slot, then out-proj pops;
                    # in the last J the backlog is scarce -- spread pops
                    slot_ctr["n"] += 1
                    if deferred:
                        fn, dJ, dh = deferred.pop(0)
                        fn()
                        if dh == GQ - 1:
                            queue_outproj(dJ)
                    if pending:
                        if len(pending) <= 8 and slot_ctr.get("lastJ") \
                                and slot_ctr["n"] % 2 == 0:
                            return
                        pending.pop(0)()

                def emit_group(st, gi, gw):
                    """one kb-group of chain st = dict(J, Jsl, h, opl, lpacc,
                    kb, nkb, ngroups)"""
                    kb = st["kb"]
                    ss = SS.tile([128, 4, JW], f32, tag="ss")
                    for i in range(gw):
                        nc.tensor.matmul(ss[:, i, :],
                                         KT[:, 0, ts(kb + i, 128)],
                                         QT[:, st["h"], st["Jsl"]],
                                         start=True, stop=True)
                    P = CP.tile([128, 4, JW], bf16, tag="p")
                    nc.scalar.activation(P[:, 0:gw, :], ss[:, 0:gw, :],
                                         AF.Exp, scale=SCALE)
                    if gi == st["ngroups"] - 1:
                        # mask the diagonal pair (last 2 kb blocks)
                        nc.vector.tensor_mul(P[:, gw - 2:gw, :],
                                             P[:, gw - 2:gw, :], msk_sb)
                    # fill the exp-latency window with deferred work
                    pop_fill()
                    # group-local L mini-sums in the score tile's second bank
                    # (closed groups, qc-major), DVE-accumulated into SBUF;
                    # the diagonal group's last block is fully masked for the
                    # lower q-half -- skip its dead contributions
                    diag = gi == st["ngroups"] - 1
                    for qc in range(2):
                        for i in range(gw):
                            if diag and i == gw - 1 and qc == 0:
                                continue
                            nc.tensor.matmul(ss[:, 3, qc:qc + 1],
                                             P[:, i, ts(qc, 128)], onesc_sb,
                                             start=(i == 0),
                                             stop=(i == gw - 1 or
                                                   (diag and qc == 0 and
                                                    i == gw - 2)))
                    if gi == 0:
                        nc.vector.tensor_copy(st["lpacc"], ss[:, 3, 0:2])
                    else:
                        nc.vector.tensor_add(st["lpacc"], st["lpacc"],
                                             ss[:, 3, 0:2])
                    for i in range(gw):
                        if diag and i == gw - 1:
                            nc.tensor.matmul(st["opl"][:, 128:JW],
                                             Vn[:, kb + i, :],
                                             P[:, i, 128:JW],
                                             start=False, stop=True,
                                             skip_group_check=True)
                        else:
                            nc.tensor.matmul(st["opl"][:, 0:JW],
                                             Vn[:, kb + i, :], P[:, i, :],
                                             start=(kb + i == 0),
                                             stop=False,
                                             skip_group_check=True)
                    st["kb"] = kb + gw

                def new_chain(J, h):
                    nkb = 2 * J + 2
                    opl = OPL.tile([128, OCH], f32, tag="opl")
                    lpacc = CT.tile([128, 2], f32, tag="lpacc")
                    groups = [4] * (nkb // 4) + ([2] if nkb % 4 else [])
                    return {"J": J, "Jsl": ts(J, JW), "h": h, "opl": opl,
                            "lpacc": lpacc, "kb": 0, "nkb": nkb,
                            "ngroups": len(groups), "groups": groups}

                # ascending J, heads interleaved in pairs
                for J in range(NJ):
                    slot_ctr["lastJ"] = (J == NJ - 1)
                    for h0 in (0, 2):
                        ca, cb = new_chain(J, h0), new_chain(J, h0 + 1)
                        for gi in range(ca["ngroups"]):
                            emit_group(ca, gi, ca["groups"][gi])
                            emit_group(cb, gi, cb["groups"][gi])
                        for st in (ca, cb):
                            deferred.append((
                                (lambda st=st: emit_lchain(
                                    st["opl"], st["lpacc"], st["h"],
                                    st["Jsl"])), st["J"], st["h"]))
                while deferred or pending:
                    pop_fill()

    nc.finalize()
    return nc


def _host_consts():
    import ml_dtypes
    bf = ml_dtypes.bfloat16
    inv = 1.0 / (ROPE_BASE ** (np.arange(0, HD, 2, dtype=np.float64) / HD))
    freqs = np.outer(np.arange(T, dtype=np.float64), inv)
    emb = np.concatenate([freqs, freqs], axis=-1)          # [T, HD]
    cosT = np.ascontiguousarray(np.cos(emb).T.astype(np.float32)).astype(bf)
    sinT = np.ascontiguousarray(np.sin(emb).T.astype(np.float32)).astype(bf)
    # diagonal pair mask: msk2[p, i*JW + q] = (128*i + p <= q)
    msk2 = np.zeros((HD, 2 * JW), np.float32)
    for i in range(2):
        k = np.arange(128)[:, None] + 128 * i
        q = np.arange(JW)[None, :]
        msk2[:, i * JW:(i + 1) * JW] = (k <= q).astype(np.float32)
    msk2 = msk2.astype(bf)
    onesn = (np.ones((128, 128), np.float32) / HD)
    onesc = np.ones((128, 1), np.float32).astype(bf)
    esel = np.zeros((2, JW), np.float32)
    for qc in range(2):
        esel[qc, qc * 128:(qc + 1) * 128] = 1.0
    esel = esel.astype(bf)
    return cosT, sinT, msk2, onesn, onesc, esel


def kernel(x, Wq, Wk, Wv, Wo, q_norm_w, k_norm_w):
    import ml_dtypes
    from concourse.bass_utils import run_bass_kernel_spmd
    bf = ml_dtypes.bfloat16

    if "nc" not in _cached:
        _cached["nc"] = _build_program()
        _cached["consts"] = _host_consts()
    nc = _cached["nc"]
    cosT, sinT, msk2, onesn, onesc, esel = _cached["consts"]

    x = np.asarray(x, np.float32)
    Wq = np.asarray(Wq, np.float32)
    Wk = np.asarray(Wk, np.float32)
    Wv = np.asarray(Wv, np.float32)
    Wo = np.asarray(Wo, np.float32)
    qwf = np.asarray(q_norm_w, np.float64).reshape(HD, 1)
    kwf = np.asarray(k_norm_w, np.float64).reshape(HD, 1)
    qw = np.ascontiguousarray((1.0 / qwf ** 2).astype(np.float32))
    kw = np.ascontiguousarray((1.0 / kwf ** 2).astype(np.float32))
    qwe = np.ascontiguousarray((EPS / qwf ** 2).astype(np.float32))
    kwe = np.ascontiguousarray((EPS / kwf ** 2).astype(np.float32))

    xTb = [np.ascontiguousarray(x[b].T).astype(bf) for b in range(B)]
    in_maps = []
    for core in range(8):
        b, kv = divmod(core, NKV)
        in_maps.append({
            "xt": xTb[b],
            "wq": np.ascontiguousarray(Wq[:, kv * HQ:(kv + 1) * HQ]).astype(bf),
            "wk": np.ascontiguousarray(Wk[:, kv * HD:(kv + 1) * HD]).astype(bf),
            "wv": np.ascontiguousarray(Wv[:, kv * HD:(kv + 1) * HD]).astype(bf),
            "wo": np.ascontiguousarray(Wo[kv * HQ:(kv + 1) * HQ, :]).astype(bf),
            "cos": cosT, "sin": sinT,
            "wqc": qw, "wkc": kw, "wqe": qwe, "wke": kwe,
            "msk2": msk2, "onesn": onesn, "onesc": onesc, "esel": esel,
        })
    res = run_bass_kernel_spmd(nc, in_maps, list(range(8)))
    out = np.zeros((B, T, D), np.float64)
    for core in range(8):
        b = core // NKV
        out[b] += res.results[core]["out"].astype(np.float64)
    return out.astype(np.float32)
